# revision 1
# baseline (speedup 1.0000x reference)
"""DogeDynamicMaskAttention Trainium2 kernel.

Sharding: 8 cores = 2 batches x 4 head-groups. Core c: batch b=c//4,
head-group g=c%4 -> heads [4g..4g+4), kv heads {2g, 2g+1}.

Device program (SPMD; identical program on all cores, different data):
  - q/k/v projections from xT as fp32r matmuls, outputs in transposed
    [out_dim, S] layout; SCALING folded into Wq on host.
  - dt = v_flat @ Wdt.T (all kv heads), dyn = exp(A * softplus(dt)).
  - exact per-head kthvalue threshold via 31-step bisection on float bits
    (dyn > 0 so float bits are monotonic; one fused DVE op per step).
  - RoPE via permutation-matmul rotate-half + DVE combine.
  - full SxS attention per head: scores psum = qk (fp32r) + rank-1 dyn-mask
    row + rank-1 causal-const row, DVE add for the non-constant (diagonal)
    mask blocks; exp with no max-subtraction (masked entries <= -1.7e38 so
    exp == 0); P * (1/l); PE transpose; attn@v; per-head output projection
    partials summed on host.
  - fully-masked (degenerate) rows give l == 0; host detects via the l
    output (and any non-finite rows) and recomputes those rows faithfully
    in numpy; expected count is ~1 row per (batch, head).
"""
import sys
import numpy as np

sys.path.insert(0, "/root/.axon_site/_ro/trn_rl_repo")

import concourse.bass as bass  # noqa: E402,F401
from concourse import bacc  # noqa: E402
import concourse.tile as tile  # noqa: E402
import concourse.mybir as mybir  # noqa: E402
from concourse.bass_utils import run_bass_kernel_spmd  # noqa: E402
from concourse.alu_op_type import AluOpType  # noqa: E402

F32 = mybir.dt.float32
F32R = mybir.dt.float32r
BF16 = mybir.dt.bfloat16
I32 = mybir.dt.int32
AF = mybir.ActivationFunctionType
AX = mybir.AxisListType.X

B, S, HID = 2, 2048, 2048
H, KV, D = 16, 8, 128
HPC, KVPC = 4, 2
GROUPS = H // KV
NUM_DYN = S // 2
SCALING = D ** -0.5
MIN = float(np.finfo(np.float32).min)
BIG = 1.7e38
P = 128
NT = S // P          # 16
NQ = 4
QW = S // NQ         # 512
NCORES = 8

_cache = {}


def _build_program(blkstate):
    key = ("nc", blkstate)
    if key in _cache:
        return _cache[key]
    nc = bacc.Bacc("TRN2", target_bir_lowering=False, debug=False,
                   num_devices=NCORES)
    dram = {}
    for name, shape in [
            ("xT", [HID, S]), ("wqT", [HID, HPC * D]), ("wkT", [HID, KVPC * D]),
            ("wvT", [HID, KVPC * D]), ("wdtvT", [HID, HPC]),
            ("woT", [HPC * D, HID]), ("acol", [HPC, 1]),
            ("cosT", [D, S]), ("sinT", [D, S]),
            ("varblk", [P, NT * P]),
            ("eye", [P, P]), ("perm", [P, P]), ("ones1", [1, P])]:
        dram[name] = nc.dram_tensor(name, shape, F32, kind="ExternalInput").ap()
    outT_d = nc.dram_tensor("outT", [HID, S], F32, kind="ExternalOutput").ap()
    dram["dyn_dr"] = nc.dram_tensor("dyn_dr", [HPC, S], F32R).ap()
    dram["vnat_dr"] = nc.dram_tensor("vnat_dr", [KVPC * NT * P, P], F32R).ap()
    l_d = nc.dram_tensor("l_out", [HPC, S], F32, kind="ExternalOutput").ap()

    with tile.TileContext(nc) as tc:
        _emit(nc, tc, dram, outT_d, l_d, blkstate)
    nc.compile()
    _cache[key] = nc
    return nc


def _emit(nc, tc, dram, outT_d, l_d, blkstate):
    from contextlib import ExitStack
    ctx = ExitStack()
    consts = ctx.enter_context(tc.tile_pool(name="consts", bufs=1))

    def cst(name, shape, src=None, as_f32r=False):
        t = consts.tile(shape, F32, name=f"c_{name}")
        nc.sync.dma_start(t[:], src if src is not None else dram[name])
        if as_f32r:
            r = consts.tile(shape, F32R, name=f"cr_{name}")
            nc.scalar.copy(r[:], t[:])
            return t, r
        return t

    eye_f, eye_r = cst("eye", [P, P], as_f32r=True)
    perm_t = cst("perm", [P, P])
    _, ones1_r = cst("ones1", [1, P], as_f32r=True)
    acol_t = cst("acol", [HPC, 1])
    # wdtvT packed [128, 16*4]: col cc*4+j = wdtvT[cc*128+p, j]
    wdtv_f = consts.tile([P, NT * HPC], F32, name="c_wdtvT")
    nc.sync.dma_start(wdtv_f[:].rearrange("p (c j) -> p c j", c=NT),
                      dram["wdtvT"].rearrange("(c p) j -> p c j", p=P))
    kthc = consts.tile([HPC, 1], F32, name="kthc")
    nc.vector.memset(kthc[:], float(NUM_DYN) - 0.5)

    act = ctx.enter_context(tc.tile_pool(name="act", bufs=1))
    qkro = [act.tile([P, S], F32R, name=f"qro{h}") for h in range(HPC)]
    kro = [act.tile([P, S], F32R, name=f"kro{i}") for i in range(KVPC)]

    with ExitStack() as ctx1:
        vop = ctx1.enter_context(tc.tile_pool(name="vop", bufs=1))
        vT_own = [vop.tile([P, S], F32R, name=f"vTown{i}") for i in range(KVPC)]
        dt_sb = vop.tile([HPC, S], F32, name="dt_sb")
        csp = ctx1.enter_context(tc.tile_pool(name="csp", bufs=1))
        cos_t = csp.tile([D, S], F32, name="cos_t")
        nc.sync.dma_start(cos_t[:], dram["cosT"])
        sin_t = csp.tile([D, S], F32, name="sin_t")
        nc.sync.dma_start(sin_t[:], dram["sinT"])

        # ---------------- dt first (enables early dyn/bisection) --------
        dyq = ctx1.enter_context(tc.tile_pool(name="dyq", bufs=1))
        with tc.tile_pool(name="dts", bufs=4) as dts, \
             tc.tile_pool(name="dps", bufs=2, space="PSUM") as dps:
            for sg in range(4):
                dtp = dps.tile([HPC, QW], F32, name="dtp", tag="dtp")
                for cc in range(NT):
                    x32 = dts.tile([P, QW], F32, name="x32", tag="x32")
                    nc.sync.dma_start(
                        x32[:], dram["xT"][cc * P:(cc + 1) * P,
                                           sg * QW:(sg + 1) * QW])
                    nc.tensor.matmul(dtp[:], wdtv_f[:, cc * HPC:(cc + 1) * HPC],
                                     x32[:], start=(cc == 0), stop=(cc == NT - 1))
                nc.scalar.copy(dt_sb[:, sg * QW:(sg + 1) * QW], dtp[:])

        # ---------------- dyn + kth bisection (overlaps projections) ----
        kth_f = dyq.tile([HPC, 1], I32, name="kth_f")
        dynrow = dyq.tile([HPC, S], F32R, name="dynrow")
        dyn_t = dyq.tile([HPC, S], F32, name="dyn_t")
        work = dyq.tile([HPC, S], F32, name="work")
        scr = dyq.tile([HPC, S], BF16, name="scr")
        scrf = dyq.tile([HPC, S], F32, name="scrf")
        nc.scalar.activation(work[:], dt_sb[:], AF.Exp)
        nc.scalar.activation(work[:], work[:], AF.Ln, bias=1.0)
        nc.scalar.activation(dyn_t[:], work[:], AF.Exp, scale=acol_t[:])
        lo = dyq.tile([HPC, 1], I32, name="lo")
        hi = dyq.tile([HPC, 1], I32, name="hi")
        mid = dyq.tile([HPC, 1], I32, name="mid")
        dlt = dyq.tile([HPC, 1], I32, name="dlt")
        cges = dyq.tile([HPC, 1], I32, name="cges")
        cltv = dyq.tile([HPC, 1], I32, name="cltv")
        cnt = dyq.tile([HPC, 1], F32, name="cnt")
        nc.vector.memset(lo[:], 0)
        nc.vector.memset(hi[:], 0x7F800000)
        for _ in range(31):
            nc.vector.tensor_tensor(dlt[:], hi[:], lo[:], op=AluOpType.subtract)
            nc.vector.tensor_scalar(dlt[:], dlt[:], 1, None,
                                    op0=AluOpType.arith_shift_right)
            nc.vector.tensor_tensor(mid[:], dlt[:], lo[:], op=AluOpType.add)
            nc.vector.tensor_scalar(scr[:], dyn_t[:],
                                    mid[:, 0:1].bitcast(F32), 0.0,
                                    op0=AluOpType.is_lt, op1=AluOpType.add,
                                    accum_out=cnt[:])
            nc.vector.tensor_scalar(cges[:], kthc[:], cnt[:, 0:1], None,
                                    op0=AluOpType.is_lt)
            nc.vector.tensor_scalar(cltv[:], kthc[:], cnt[:, 0:1], None,
                                    op0=AluOpType.is_ge)
            nc.vector.copy_predicated(hi[:], cges[:], mid[:])
            nc.vector.copy_predicated(lo[:], cltv[:], mid[:])
        nc.vector.tensor_copy(kth_f[:], lo[:])
        pen = scrf
        nc.vector.tensor_scalar(pen[:], dyn_t[:],
                                kth_f[:, 0:1].bitcast(F32), -BIG,
                                op0=AluOpType.is_lt, op1=AluOpType.mult)
        nc.vector.tensor_tensor(dynrow[:], dyn_t[:], pen[:], op=AluOpType.add)
        nc.sync.dma_start(dram["dyn_dr"], dynrow[:])

        # ---------------- projections ----------------
        with tc.tile_pool(name="xp", bufs=1) as xp, \
             tc.tile_pool(name="wp", bufs=2) as wp, \
             tc.tile_pool(name="pjp", bufs=5) as pjp, \
             tc.tile_pool(name="pps", bufs=8, space="PSUM") as pps:
            wname = {"v": "wvT", "q": "wqT", "k": "wkT"}
            OT = ([("v", i) for i in range(KVPC)]
                  + [("q", i) for i in range(HPC)]
                  + [("k", i) for i in range(KVPC)])
            for sg in range(4):
                xfull = xp.tile([P, NT * QW], F32R, name="xfull", tag="xf")
                nc.gpsimd.dma_start(
                    xfull[:].rearrange("p (c f) -> p c f", c=NT),
                    dram["xT"][:, sg * QW:(sg + 1) * QW]
                    .rearrange("(c p) f -> p c f", p=P))
                for kind, oi in OT:
                    wfull = wp.tile([P, NT * P], F32R, name="wfull", tag="wf")
                    nc.gpsimd.dma_start(
                        wfull[:].rearrange("p (c f) -> p c f", c=NT),
                        dram[wname[kind]][:, oi * P:(oi + 1) * P]
                        .rearrange("(c p) f -> p c f", p=P))
                    ps = pps.tile([P, QW], F32, name="ps", tag="ps")
                    for cc in range(NT):
                        nc.tensor.matmul(ps[:], wfull[:, cc * P:(cc + 1) * P],
                                         xfull[:, cc * QW:(cc + 1) * QW],
                                         start=(cc == 0), stop=(cc == NT - 1))
                    if kind == "v":
                        dst = vT_own[oi][:, sg * QW:(sg + 1) * QW]
                        nc.scalar.copy(dst, ps[:])
                    else:
                        f32t = pjp.tile([P, QW], F32, name="pj32", tag="pj")
                        nc.scalar.copy(f32t[:], ps[:])
                        dstro = (qkro[oi] if kind == "q" else kro[oi])
                        rh = pps.tile([P, QW], F32, name="rh", tag="ps")
                        nc.tensor.matmul(rh[:], perm_t[:], f32t[:],
                                         start=True, stop=True)
                        t1 = pjp.tile([P, QW], F32, name="t1", tag="pj")
                        nc.vector.tensor_tensor(
                            t1[:], rh[:], sin_t[:, sg * QW:(sg + 1) * QW],
                            op=AluOpType.mult)
                        t2 = pjp.tile([P, QW], F32, name="t2", tag="pj")
                        nc.vector.tensor_tensor(
                            t2[:], f32t[:], cos_t[:, sg * QW:(sg + 1) * QW],
                            op=AluOpType.mult)
                        nc.vector.tensor_tensor(
                            dstro[:, sg * QW:(sg + 1) * QW], t1[:], t2[:],
                            op=AluOpType.add)

        # ---------------- natural-layout v tiles (bounced via DRAM) ------
        with tc.tile_pool(name="vnb", bufs=4) as vnb, \
             tc.tile_pool(name="vps", bufs=4, space="PSUM") as vps:
            for i in range(KVPC):
                for cc in range(NT):
                    pt = vps.tile([P, P], F32, name="vt", tag="vt")
                    nc.tensor.transpose(pt[:].bitcast(F32R),
                                        vT_own[i][:, cc * P:(cc + 1) * P],
                                        eye_r[:])
                    vn = vnb.tile([P, P], F32R, name="vn", tag="vn")
                    nc.scalar.copy(vn[:], pt[:])
                    nc.sync.dma_start(
                        dram["vnat_dr"][(i * NT + cc) * P:(i * NT + cc + 1) * P, :],
                        vn[:])

    # ---------------- attention ----------------
    # blkstate[t][j] in {"Z", "M", "V:<idx>"}: zero / masked-const / varying
    # computed extent per tile: up to last non-M block
    ext = []
    for t in range(NT):
        nz = [j for j in range(NT) if blkstate[t][j] != "M"]
        ext.append((max(nz) + 1) * P if nz else 0)
    ares = ctx.enter_context(tc.tile_pool(name="ares", bufs=1))
    attnT = [ares.tile([P, S], F32R, name=f"attnT{h}") for h in range(HPC)]
    dynrow0 = [ares.tile([1, S], F32R, name=f"dynrow0_{h}") for h in range(HPC)]
    varblk_t = ares.tile([P, NT * P], F32, name="varblk_t")
    nc.sync.dma_start(varblk_t[:], dram["varblk"])
    for h in range(HPC):
        nc.sync.dma_start(dynrow0[h][:], dram["dyn_dr"][h:h + 1, :])
    with tc.tile_pool(name="ppl", bufs=6) as ppl, \
         tc.tile_pool(name="lpl", bufs=16) as lpl, \
         tc.tile_pool(name="ptl", bufs=6) as ptl, \
         tc.tile_pool(name="vnl", bufs=8) as vnl, \
         tc.tile_pool(name="aps", bufs=6, space="PSUM") as aps, \
         tc.tile_pool(name="ovl", bufs=2, space="PSUM") as ovl:
        for h in range(HPC):
            kv = h // GROUPS
            for grp in range(4):
                glim = max(ext[grp * 4 + tq] for tq in range(4))
                glim = ((glim + QW - 1) // QW) * QW  # pad group extent to 512
                ptiles = []
                for tq in range(4):
                    t = grp * 4 + tq
                    ptile = ppl.tile([P, S], F32R, name="ptile", tag="pt")
                    lparts = lpl.tile([P, NQ], F32, name="lparts", tag="lp")
                    nc.vector.memset(lparts[:], 0.0)
                    for qq in range(NQ):
                        q0 = qq * QW
                        e = min(max(ext[t] - q0, 0), QW)
                        if q0 >= glim:
                            break  # rest of group never read
                        if e == 0:
                            nc.vector.memset(ptile[:, q0:min(q0 + QW, glim)].bitcast(F32), 0.0)
                            nc.vector.memset(lparts[:, qq:qq + 1], 0.0)
                            continue
                        sc = aps.tile([P, QW], F32, name="sc", tag="aps")
                        nc.tensor.matmul(
                            sc[:, :e], qkro[h][:, t * P:(t + 1) * P],
                            kro[kv][:, q0:q0 + e],
                            start=True, stop=True, skip_group_check=True)
                        nc.tensor.matmul(
                            sc[:, :e], ones1_r[:], dynrow0[h][:, q0:q0 + e],
                            start=False, stop=True, skip_group_check=True)
                        for j in range(q0 // P, (q0 + e) // P):
                            st = blkstate[t][j]
                            if st.startswith("V"):
                                vi = int(st[2:])
                                off = j * P - q0
                                nc.vector.tensor_tensor(
                                    sc[:, off:off + P], sc[:, off:off + P],
                                    varblk_t[:, vi * P:(vi + 1) * P],
                                    op=AluOpType.add)
                        nc.scalar.activation(
                            ptile[:, q0:q0 + e], sc[:, :e], AF.Exp,
                            accum_out=lparts[:, qq:qq + 1])
                        if e < QW and q0 + e < glim:
                            nc.vector.memset(
                                ptile[:, q0 + e:min(q0 + QW, glim)]
                                .bitcast(F32), 0.0)
                    lsum = lpl.tile([P, 1], F32, name="lsum", tag="ls")
                    nc.vector.reduce_sum(lsum[:], lparts[:], axis=AX)
                    nc.sync.dma_start(
                        l_d[h:h + 1, t * P:(t + 1) * P].rearrange("a b -> b a"),
                        lsum[:])
                    linv = lpl.tile([P, 1], F32, name="linv", tag="ls")
                    nc.vector.reciprocal(linv[:], lsum[:])
                    nc.vector.tensor_scalar(ptile[:, :glim], ptile[:, :glim],
                                            linv[:, 0:1],
                                            None, op0=AluOpType.mult)
                    ptiles.append(ptile)
                ovp = ovl.tile([P, QW], F32, name="ovp", tag="ovp")
                nch = glim // P
                for cc in range(nch):
                    ptt = aps.tile([P, QW], F32, name="ptt", tag="aps")
                    for tq in range(4):
                        nc.tensor.transpose(
                            ptt[:, tq * P:(tq + 1) * P].bitcast(F32R),
                            ptiles[tq][:, cc * P:(cc + 1) * P], eye_r[:])
                    pts = ptl.tile([P, QW], F32R, name="pts", tag="pts")
                    nc.vector.tensor_copy(pts[:], ptt[:])
                    vn = vnl.tile([P, P], F32R, name="vnt", tag="vnt")
                    nc.sync.dma_start(
                        vn[:], dram["vnat_dr"]
                        [(kv * NT + cc) * P:(kv * NT + cc + 1) * P, :])
                    nc.tensor.matmul(ovp[:], vn[:], pts[:],
                                     start=(cc == 0), stop=(cc == nch - 1),
                                     skip_group_check=True)
                nc.scalar.copy(attnT[h][:, grp * QW:(grp + 1) * QW], ovp[:])

    # ---------------- output projection ----------------
    with tc.tile_pool(name="wol", bufs=2) as wol, \
         tc.tile_pool(name="oub", bufs=4) as oub, \
         tc.tile_pool(name="ops", bufs=4, space="PSUM") as ops:
        for ht in range(NT):
            wo = wol.tile([P, HPC * P], F32R, name="wo", tag="wo")
            nc.gpsimd.dma_start(
                wo[:].rearrange("p (h f) -> p h f", h=HPC),
                dram["woT"][:, ht * P:(ht + 1) * P]
                .rearrange("(h p) f -> p h f", p=P))
            for sg in range(4):
                op = ops.tile([P, QW], F32, name="op", tag="op")
                for h in range(HPC):
                    nc.tensor.matmul(op[:], wo[:, h * P:(h + 1) * P],
                                     attnT[h][:, sg * QW:(sg + 1) * QW],
                                     start=(h == 0), stop=(h == HPC - 1))
                ot = oub.tile([P, QW], F32, name="ot", tag="ot")
                nc.scalar.copy(ot[:], op[:])
                nc.sync.dma_start(
                    outT_d[ht * P:(ht + 1) * P, sg * QW:(sg + 1) * QW], ot[:])
    ctx.close()


def _host_prep(hidden_states, cos, sin, attention_mask, Wq, Wk, Wv, A, Wdt, Wo):
    eye = np.eye(P, dtype=np.float32)
    perm = np.zeros((P, P), dtype=np.float32)
    for j in range(64):
        perm[j + 64, j] = -1.0
        perm[j, j + 64] = 1.0
    ones1 = np.ones((1, P), dtype=np.float32)

    in_maps = []
    blkstates = []
    for c in range(NCORES):
        b, g = divmod(c, 4)
        heads = list(range(4 * g, 4 * g + 4))
        wvT = np.ascontiguousarray(Wv[2 * g * D:(2 * g + 2) * D].T)
        wdtvT = np.ascontiguousarray(
            (Wdt[heads].astype(np.float64) @ Wv.astype(np.float64))
            .T.astype(np.float32))
        xT = np.ascontiguousarray(hidden_states[b].T)
        wqT = np.ascontiguousarray(
            (Wq[4 * g * D:(4 * g + 4) * D] * np.float32(SCALING)).T)
        wkT = np.ascontiguousarray(Wk[2 * g * D:(2 * g + 2) * D].T)
        woT = np.ascontiguousarray(Wo[:, 4 * g * D:(4 * g + 4) * D].T)
        acol = A[heads].astype(np.float32).reshape(HPC, 1)
        cosT = np.ascontiguousarray(cos[b].T)
        sinT = np.ascontiguousarray(sin[b].T)
        m = attention_mask[b, 0]
        mb = m.reshape(NT, P, NT, P)
        blkrows = []
        varlist = []
        for t in range(NT):
            row = []
            for j in range(NT):
                blkv = mb[t, :, j, :]
                if np.all(blkv == 0):
                    row.append("Z")
                elif np.all(blkv <= -1e30):
                    row.append("M")
                else:
                    row.append(f"V:{len(varlist)}")
                    varlist.append(np.maximum(blkv, -BIG))
            # interior M blocks (before a later non-M block) become varying
            nz = [j for j in range(NT) if row[j] != "M"]
            lim = (max(nz) + 1) if nz else 0
            for j in range(lim):
                if row[j] == "M":
                    row[j] = f"V:{len(varlist)}"
                    varlist.append(np.full((P, P), -BIG, np.float32))
            blkrows.append(tuple(row))
        if len(varlist) > NT:
            raise NotImplementedError("too many varying mask blocks")
        varblk = np.zeros((P, NT * P), dtype=np.float32)
        for vi, blkv in enumerate(varlist):
            varblk[:, vi * P:(vi + 1) * P] = blkv
        blkstate = tuple(blkrows)
        in_maps.append({
            "xT": xT, "wqT": wqT, "wkT": wkT, "wvT": wvT, "wdtvT": wdtvT,
            "woT": woT, "acol": acol, "cosT": cosT, "sinT": sinT,
            "varblk": varblk, "eye": eye, "perm": perm,
            "ones1": ones1,
        })
        blkstates.append(blkstate)
    if len(set(blkstates)) != 1:
        raise NotImplementedError("mask structure differs across batches")
    return in_maps, blkstates[0]


def _softplus64(x):
    x = x.astype(np.float64)
    return np.log1p(np.exp(-np.abs(x))) + np.maximum(x, 0)


def _repair_rows(out, bad, inputs):
    """Recompute rows flagged bad [B, S] with faithful numpy reference math."""
    if not bad.any():
        return out
    hs = inputs["hidden_states"]; cos = inputs["cos"]; sin = inputs["sin"]
    am = inputs["attention_mask"]; Wq = inputs["Wq"]; Wk = inputs["Wk"]
    Wv = inputs["Wv"]; A = inputs["A"]; Wdt = inputs["Wdt"]; Wo = inputs["Wo"]

    def rope(x, c, s):
        x1, x2 = x[..., :D // 2], x[..., D // 2:]
        return x * c + np.concatenate([-x2, x1], axis=-1) * s

    for b in range(B):
        rows = np.where(bad[b])[0]
        if len(rows) == 0:
            continue
        x = hs[b].astype(np.float32)
        k = (x @ Wk.T).reshape(S, KV, D)
        v = (x @ Wv.T).reshape(S, KV, D)
        k = rope(k, cos[b][:, None, :], sin[b][:, None, :])
        v_flat = v.reshape(S, KV * D)
        dt = v_flat @ Wdt.T
        dyn = np.exp(A[None, :] * _softplus64(dt)).astype(np.float32).T
        kth = np.sort(dyn, axis=-1)[:, NUM_DYN - 1:NUM_DYN]
        dmask = np.where(dyn < kth, MIN, dyn).astype(np.float32)
        for s_i in rows:
            q_row = (x[s_i] @ Wq.T).reshape(H, D)
            q_row = rope(q_row, cos[b][s_i][None, :], sin[b][s_i][None, :])
            attn_row = np.zeros((H, D), dtype=np.float32)
            for h in range(H):
                kvh = h // GROUPS
                sc = ((q_row[h] @ k[:, kvh].T) * np.float32(SCALING)
                      + (dmask[h] + am[b, 0, s_i])).astype(np.float32)
                w = np.exp(sc - sc.max())
                w = (w / w.sum()).astype(np.float32)
                attn_row[h] = w @ v[:, kvh]
            out[b, s_i] = attn_row.reshape(H * D) @ Wo.T
    return out


def kernel(**inputs):
    inputs = {k: np.asarray(v) for k, v in inputs.items()}
    in_maps, blkstate = _host_prep(**inputs)
    nc = _build_program(blkstate)
    res = run_bass_kernel_spmd(nc, in_maps, list(range(NCORES)))
    out = np.zeros((B, S, HID), dtype=np.float32)
    bad = np.zeros((B, S), dtype=bool)
    for c in range(NCORES):
        b = c // 4
        out[b] += res.results[c]["outT"].T
        bad[b] |= (res.results[c]["l_out"] == 0).any(axis=0)
    bad |= ~np.isfinite(out).all(axis=2)
    out = _repair_rows(out, bad, inputs)
    return out



# revision 17
# speedup vs baseline: 1.5520x; 1.5520x over previous
"""DogeDynamicMaskAttention Trainium2 kernel (v3).

Sharding: 8 cores = 2 batches x 4 head-groups. Core c: batch b=c//4,
head-group g=c%4 -> heads [4g..4g+4), kv heads {2g, 2g+1}.

Device program (SPMD; identical program on all cores, different data):
  - dt = (Wdt@Wv) @ x as an f32r pre-pass (the kthvalue threshold needs
    near-fp32 dt; bf16 dt flips mask membership and fails the rel-err
    gate). The bf16 x working set is derived on-device from the same f32
    stream (scalar copies), so x is DMA'd once.
  - q/k/v projections in bf16 (validated ~5e-3 rel err), PSUM fp32;
    per-output stationary reused across 4 seq blocks.
  - dyn = exp(A*softplus(dt)); exact per-head kthvalue via float-bit
    bisection (device min/max init, 27 steps); penalized row transposed
    to a [keys, head] column layout via PE transposes.
  - attention computed TRANSPOSED: scores^T [keys, q] = kro_kc^T @ qkro.
    The dynamic mask is a per-partition (per-key) bias fused into the exp
    activation; P^T comes out directly so attn@v needs no P transposes.
    l (softmax denom) via ones-column matmuls; 1/l broadcast across
    partitions via a rank-1 f32r matmul; tails (broadcast + normalize)
    are software-pipelined one unit behind so the Tensor queue never
    waits on the DVE reciprocal.
  - output projection interleaved per q-block round (starts as soon as
    all heads finished that q range), Wo resident in SBUF.
  - causal handling: per key-chunk q-extent; diagonal [128,128] blocks
    get a transposed additive mask; exp with no max-subtraction (masked
    entries <= -1.7e38 so exp == 0).
  - per-head output projection partials summed on host.
  - fully-masked (degenerate) q rows give l == 0; host detects via the l
    output (and any non-finite rows) and recomputes those rows faithfully
    in numpy; expected count is ~1 row per (batch, head).
"""
import sys
import numpy as np
import ml_dtypes

sys.path.insert(0, "/root/.axon_site/_ro/trn_rl_repo")

import concourse.bass as bass  # noqa: E402,F401
from concourse import bacc  # noqa: E402
import concourse.tile as tile  # noqa: E402
import concourse.mybir as mybir  # noqa: E402
from concourse.bass_utils import run_bass_kernel_spmd  # noqa: E402
from concourse.alu_op_type import AluOpType  # noqa: E402

F32 = mybir.dt.float32
F32R = mybir.dt.float32r
BF16 = mybir.dt.bfloat16
I32 = mybir.dt.int32
AF = mybir.ActivationFunctionType
AX = mybir.AxisListType.X

B, S, HID = 2, 2048, 2048
H, KV, D = 16, 8, 128
HPC, KVPC = 4, 2
GROUPS = H // KV
NUM_DYN = S // 2
SCALING = D ** -0.5
MIN = float(np.finfo(np.float32).min)
BIG = 1.7e38
P = 128
NT = S // P          # 16
NQ = 4
QW = S // NQ         # 512
NCORES = 8
NBIS = 27
BF = ml_dtypes.bfloat16

_cache = {}


def _build_program(blkkey):
    key = ("nc", blkkey)
    if key in _cache:
        return _cache[key]
    nc = bacc.Bacc("TRN2", target_bir_lowering=False, debug=False,
                   num_devices=NCORES)
    dram = {}
    for name, shape, dt in [
            ("xT", [HID, S], F32), ("wq", [HID, HPC * D], BF16),
            ("wk", [HID, KVPC * D], BF16), ("wv", [HID, KVPC * D], BF16),
            ("wdtv", [HID, HPC], F32), ("wo", [HPC * D, HID], BF16),
            ("acol", [HPC, 1], F32), ("cosT", [D, S], BF16),
            ("sinT", [D, S], BF16), ("varblkT", [P, NT * P], BF16),
            ("eye_bf", [P, P], BF16), ("eye4", [HPC, HPC], F32),
            ("perm_bf", [P, P], BF16), ("ones1", [1, P], F32),
            ("onescol", [P, 1], BF16)]:
        dram[name] = nc.dram_tensor(name, shape, dt, kind="ExternalInput").ap()
    outT_d = nc.dram_tensor("outT", [HID, S], F32, kind="ExternalOutput").ap()
    l_d = nc.dram_tensor("l_out", [HPC, S], F32, kind="ExternalOutput").ap()

    blkT, qmin_t = blkkey
    with tile.TileContext(nc) as tc:
        _emit(nc, tc, dram, outT_d, l_d, blkT, qmin_t)
    nc.compile()
    _cache[key] = nc
    return nc


def _emit(nc, tc, dram, outT_d, l_d, blkT, qmin_t):
    from contextlib import ExitStack
    ctx = ExitStack()
    consts = ctx.enter_context(tc.tile_pool(name="consts", bufs=1))

    def cst(name, shape, dt):
        t = consts.tile(shape, dt, name=f"c_{name}")
        nc.sync.dma_start(t[:], dram[name])
        return t

    eye_bf = cst("eye_bf", [P, P], BF16)
    eye4_t = cst("eye4", [HPC, HPC], F32)
    perm_t = cst("perm_bf", [P, P], BF16)
    ones1_t = cst("ones1", [1, P], F32)
    ones1_r = consts.tile([1, P], F32R, name="ones1_r")
    nc.scalar.copy(ones1_r[:], ones1_t[:])
    onescol_t = cst("onescol", [P, 1], BF16)
    acol_t = cst("acol", [HPC, 1], F32)
    varblk_t = cst("varblkT", [P, NT * P], BF16)
    cos_t = cst("cosT", [D, S], BF16)
    sin_t = cst("sinT", [D, S], BF16)
    # wdtv packed f32r [128, 16*4]: col cc*4+j = wdtv[cc*128+p, j]
    wdtv_f = consts.tile([P, NT * HPC], F32R, name="c_wdtv")
    nc.sync.dma_start(wdtv_f[:].rearrange("p (c j) -> p c j", c=NT),
                      dram["wdtv"].bitcast(F32R)
                      .rearrange("(c p) j -> p c j", p=P))
    kthc = consts.tile([HPC, 1], F32, name="kthc")
    nc.vector.memset(kthc[:], float(NUM_DYN) - 0.5)

    act = ctx.enter_context(tc.tile_pool(name="act", bufs=1))
    qkro = [act.tile([P, S], BF16, name=f"qro{h}") for h in range(HPC)]
    kro = [act.tile([P, S], BF16, name=f"kro{i}") for i in range(KVPC)]
    vn = act.tile([P, KVPC * NT * P], BF16, name="vn")
    attnT = [act.tile([P, S], BF16, name=f"attnT{h}") for h in range(HPC)]
    dyncol = act.tile([P, NT * HPC], F32, name="dyncol")
    dynrow = act.tile([HPC, S], F32, name="dynrow")

    with ExitStack() as ctx1:
        xp = ctx1.enter_context(tc.tile_pool(name="xp", bufs=1))
        xfull = xp.tile([P, NT * S], BF16, name="xfull")
        vT = [xp.tile([P, S], BF16, name=f"vT{i}") for i in range(KVPC)]
        dt_sb = xp.tile([HPC, S], F32, name="dt_sb")
        x3p = ctx1.enter_context(tc.tile_pool(name="x3p", bufs=2))
        dyq = ctx1.enter_context(tc.tile_pool(name="dyq", bufs=1))
        wp = ctx1.enter_context(tc.tile_pool(name="wp", bufs=2))
        pjp = ctx1.enter_context(tc.tile_pool(name="pjp", bufs=4))
        rsb = ctx1.enter_context(tc.tile_pool(name="rsb", bufs=3))

        # ---------------- dt pre-pass (f32r; x streamed once) ------------
        with tc.tile_pool(name="dps", bufs=4, space="PSUM") as dps:
            dt_ps = [dps.tile([HPC, QW], F32, name="dtp", tag="dtp")
                     for _ in range(NQ)]
            for cc in range(NT):
                x32 = x3p.tile([P, S], F32R, name="x32", tag="x32")
                nc.sync.dma_start(
                    x32[:], dram["xT"].bitcast(F32R)[cc * P:(cc + 1) * P, :])
                for sg in range(NQ):
                    nc.tensor.matmul(
                        dt_ps[sg][:], wdtv_f[:, cc * HPC:(cc + 1) * HPC],
                        x32[:, sg * QW:(sg + 1) * QW],
                        start=(cc == 0), stop=(cc == NT - 1),
                        skip_group_check=True)
                nc.scalar.copy(xfull[:, cc * S:(cc + 1) * S],
                               x32[:].bitcast(F32))
            for sg in range(NQ):
                nc.scalar.copy(dt_sb[:, sg * QW:(sg + 1) * QW], dt_ps[sg][:])

        # ---------------- dyn + kth bisection (DVE; overlaps proj) -------
        kth_f = dyq.tile([HPC, 1], I32, name="kth_f")
        dyn_t = dyq.tile([HPC, S], F32, name="dyn_t")
        work = dyq.tile([HPC, S], F32, name="work")
        scr = dyq.tile([HPC, S], BF16, name="scr")
        nc.scalar.activation(work[:], dt_sb[:], AF.Exp)
        nc.scalar.activation(work[:], work[:], AF.Ln, bias=1.0)
        nc.scalar.activation(dyn_t[:], work[:], AF.Exp, scale=acol_t[:])
        mn = dyq.tile([HPC, 1], F32, name="mn")
        mx = dyq.tile([HPC, 1], F32, name="mx")
        nc.vector.tensor_reduce(mn[:], dyn_t[:], axis=AX, op=AluOpType.min)
        nc.vector.tensor_reduce(mx[:], dyn_t[:], axis=AX, op=AluOpType.max)
        lo = dyq.tile([HPC, 1], I32, name="lo")
        hi = dyq.tile([HPC, 1], I32, name="hi")
        mid = dyq.tile([HPC, 1], I32, name="mid")
        dlt = dyq.tile([HPC, 1], I32, name="dlt")
        cges = dyq.tile([HPC, 1], I32, name="cges")
        cltv = dyq.tile([HPC, 1], I32, name="cltv")
        cnt = dyq.tile([HPC, 1], F32, name="cnt")
        nc.vector.tensor_copy(lo[:], mn[:].bitcast(I32))
        nc.vector.tensor_scalar(hi[:], mx[:].bitcast(I32), 1, None,
                                op0=AluOpType.add)
        for _ in range(NBIS):
            nc.vector.tensor_tensor(dlt[:], hi[:], lo[:], op=AluOpType.subtract)
            nc.vector.tensor_scalar(dlt[:], dlt[:], 1, None,
                                    op0=AluOpType.arith_shift_right)
            nc.vector.tensor_tensor(mid[:], dlt[:], lo[:], op=AluOpType.add)
            nc.vector.tensor_scalar(scr[:], dyn_t[:],
                                    mid[:, 0:1].bitcast(F32), 0.0,
                                    op0=AluOpType.is_lt, op1=AluOpType.add,
                                    accum_out=cnt[:])
            nc.vector.tensor_scalar(cges[:], kthc[:], cnt[:, 0:1],
                                    None, op0=AluOpType.is_lt)
            nc.vector.tensor_scalar(cltv[:], kthc[:], cnt[:, 0:1],
                                    None, op0=AluOpType.is_ge)
            nc.vector.copy_predicated(hi[:], cges[:], mid[:])
            nc.vector.copy_predicated(lo[:], cltv[:], mid[:])
        nc.vector.tensor_copy(kth_f[:], lo[:])
        pen = work
        nc.vector.tensor_scalar(pen[:], dyn_t[:],
                                kth_f[:, 0:1].bitcast(F32), -BIG,
                                op0=AluOpType.is_lt, op1=AluOpType.mult)
        nc.vector.tensor_tensor(dynrow[:], dyn_t[:], pen[:],
                                op=AluOpType.add)

        # ---------------- projections (bf16) -----------------------------
        with tc.tile_pool(name="pps", bufs=6, space="PSUM") as pps, \
             tc.tile_pool(name="rps", bufs=2, space="PSUM") as rps:
            wname = {"v": "wv", "q": "wq", "k": "wk"}
            OT = ([("v", i) for i in range(KVPC)]
                  + [("k", i) for i in range(KVPC)]
                  + [("q", i) for i in range(HPC)])
            for kind, oi in OT:
                wfull = wp.tile([P, NT * P], BF16, name="wfull", tag="wf")
                nc.gpsimd.dma_start(
                    wfull[:].rearrange("p (c f) -> p c f", c=NT),
                    dram[wname[kind]][:, oi * P:(oi + 1) * P]
                    .rearrange("(c p) f -> p c f", p=P))
                ps_sg = [pps.tile([P, QW], F32, name="ps", tag="ps")
                         for _ in range(NQ)]
                for cc in range(NT):
                    st = wfull[:, cc * P:(cc + 1) * P]
                    for sg in range(NQ):
                        nc.tensor.matmul(
                            ps_sg[sg][:], st,
                            xfull[:, cc * S + sg * QW:cc * S + (sg + 1) * QW],
                            start=(cc == 0), stop=(cc == NT - 1),
                            skip_group_check=True)
                if kind == "v":
                    for sg in range(NQ):
                        nc.scalar.copy(vT[oi][:, sg * QW:(sg + 1) * QW],
                                       ps_sg[sg][:])
                else:
                    dstro = qkro[oi] if kind == "q" else kro[oi]
                    for sg in range(NQ):
                        q_sb = pjp.tile([P, QW], BF16, name="q_sb", tag="pj")
                        nc.scalar.copy(q_sb[:], ps_sg[sg][:])
                        rh = rps.tile([P, QW], F32, name="rh", tag="rh")
                        nc.tensor.matmul(rh[:], perm_t[:], q_sb[:],
                                         start=True, stop=True,
                                         skip_group_check=True)
                        rh_sb = rsb.tile([P, QW], BF16, name="rh_sb", tag="rs")
                        nc.scalar.copy(rh_sb[:], rh[:])
                        t1 = pjp.tile([P, QW], BF16, name="t1", tag="pj")
                        nc.vector.tensor_tensor(
                            t1[:], rh_sb[:], sin_t[:, sg * QW:(sg + 1) * QW],
                            op=AluOpType.mult)
                        t2 = pjp.tile([P, QW], BF16, name="t2", tag="pj")
                        nc.vector.tensor_tensor(
                            t2[:], q_sb[:], cos_t[:, sg * QW:(sg + 1) * QW],
                            op=AluOpType.mult)
                        nc.vector.tensor_tensor(
                            dstro[:, sg * QW:(sg + 1) * QW], t1[:], t2[:],
                            op=AluOpType.add)

        # v natural-layout + dyncol transposes (proj PSUM pools closed)
        with tc.tile_pool(name="tps", bufs=4, space="PSUM") as tps:
            for i in range(KVPC):
                for cc in range(NT):
                    pt = tps.tile([P, P], BF16, name="vt", tag="vt")
                    nc.tensor.transpose(pt[:], vT[i][:, cc * P:(cc + 1) * P],
                                        eye_bf[:])
                    nc.scalar.copy(
                        vn[:, (i * NT + cc) * P:(i * NT + cc + 1) * P], pt[:])
            for cc in range(NT):
                dc = tps.tile([P, HPC], F32, name="dc", tag="dc")
                nc.tensor.transpose(dc[:], dynrow[:, cc * P:(cc + 1) * P],
                                    eye4_t[:])
                nc.scalar.copy(dyncol[:, cc * HPC:(cc + 1) * HPC], dc[:])

    # ---------------- attention + interleaved output projection ---------
    with tc.tile_pool(name="wop", bufs=1) as wop, \
         tc.tile_pool(name="ptp", bufs=18) as ptp, \
         tc.tile_pool(name="lsp", bufs=3) as lsp, \
         tc.tile_pool(name="oub", bufs=3) as oub, \
         tc.tile_pool(name="scp", bufs=2, space="PSUM") as scp, \
         tc.tile_pool(name="ovl", bufs=2, space="PSUM") as ovl, \
         tc.tile_pool(name="lpp", bufs=2, space="PSUM") as lpp, \
         tc.tile_pool(name="bcp", bufs=1, space="PSUM") as bcp, \
         tc.tile_pool(name="opp", bufs=1, space="PSUM") as opp:
        wo_all = wop.tile([P, NT * HPC * P], BF16, name="wo_all")
        nc.sync.dma_start(
            wo_all[:].rearrange("p (t h f) -> p t h f", t=NT, h=HPC),
            dram["wo"].rearrange("(h p) (t f) -> p t h f", p=P, f=P))

        def emit_unit(h, qb):
            kv = h // GROUPS
            kcs = [kc for kc in range(NT) if qmin_t[kc] < 4 * (qb + 1)]
            pts = []
            for i, kc in enumerate(kcs):
                offt = max(qmin_t[kc] - 4 * qb, 0)
                off = offt * P
                assert i > 0 or off == 0
                sc = scp.tile([P, QW], F32, name="sc", tag="sc")
                nc.tensor.matmul(
                    sc[:, off:QW],
                    kro[kv][:, kc * P:(kc + 1) * P],
                    qkro[h][:, qb * QW + off:(qb + 1) * QW],
                    start=True, stop=True, skip_group_check=True)
                for t in range(4 * qb + offt, 4 * qb + 4):
                    st = blkT[kc][t]
                    if st.startswith("V"):
                        vi = int(st[2:])
                        o2 = (t - 4 * qb) * P
                        nc.vector.tensor_tensor(
                            sc[:, o2:o2 + P], sc[:, o2:o2 + P],
                            varblk_t[:, vi * P:(vi + 1) * P],
                            op=AluOpType.add)
                pt = ptp.tile([P, QW], BF16, name="pt", tag="pt")
                nc.scalar.activation(
                    pt[:, off:QW], sc[:, off:QW], AF.Exp,
                    bias=dyncol[:, kc * HPC + h:kc * HPC + h + 1])
                pts.append((kc, pt, off))
            lp = lpp.tile([1, QW], F32, name="lp", tag="lp")
            for i, (kc, pt, off) in enumerate(pts):
                nc.tensor.matmul(lp[:, off:QW], onescol_t[:], pt[:, off:QW],
                                 start=(i == 0), stop=(i == len(pts) - 1),
                                 skip_group_check=True)
            lsb = lsp.tile([1, QW], F32, name="lsb", tag="lsb")
            nc.vector.tensor_copy(lsb[:], lp[:])
            nc.sync.dma_start(l_d[h:h + 1, qb * QW:(qb + 1) * QW], lsb[:])
            linv = lsp.tile([1, QW], F32, name="linv", tag="li")
            nc.vector.reciprocal(linv[:], lp[:])
            linv_r = lsp.tile([1, QW], F32R, name="linv_r", tag="lir")
            nc.scalar.copy(linv_r[:], linv[:])
            ovp = ovl.tile([P, QW], F32, name="ovp", tag="ovp")
            for i, (kc, pt, off) in enumerate(pts):
                nc.tensor.matmul(
                    ovp[:, off:QW],
                    vn[:, (kv * NT + kc) * P:(kv * NT + kc + 1) * P],
                    pt[:, off:QW],
                    start=(i == 0), stop=(i == len(kcs) - 1),
                    skip_group_check=True)
            return (h, qb, ovp, linv_r)

        def emit_tail(st):
            h, qb, ovp, linv_r = st
            bc = bcp.tile([P, QW], F32, name="bc", tag="bc")
            nc.tensor.matmul(bc[:], ones1_r[:], linv_r[:],
                             start=True, stop=True, skip_group_check=True)
            bcs = lsp.tile([P, QW], F32, name="bcs", tag="bcs")
            nc.scalar.copy(bcs[:], bc[:])
            nc.vector.tensor_tensor(
                attnT[h][:, qb * QW:(qb + 1) * QW], ovp[:], bcs[:],
                op=AluOpType.mult)

        def emit_outproj(sg):
            for ht in range(NT):
                op_ps = opp.tile([P, QW], F32, name="op", tag="op")
                for h in range(HPC):
                    nc.tensor.matmul(
                        op_ps[:], wo_all[:, (ht * HPC + h) * P:
                                         (ht * HPC + h + 1) * P],
                        attnT[h][:, sg * QW:(sg + 1) * QW],
                        start=(h == 0), stop=(h == HPC - 1),
                        skip_group_check=True)
                ot = oub.tile([P, QW], F32, name="ot", tag="ot")
                nc.scalar.copy(ot[:], op_ps[:])
                nc.sync.dma_start(
                    outT_d[ht * P:(ht + 1) * P, sg * QW:(sg + 1) * QW], ot[:])

        pending = None
        for qb in range(NQ):
            for h in range(HPC):
                st = emit_unit(h, qb)
                if pending is not None:
                    emit_tail(pending)
                pending = st
            if qb > 0:
                emit_outproj(qb - 1)
        emit_tail(pending)
        emit_outproj(NQ - 1)
    ctx.close()


def _host_prep(hidden_states, cos, sin, attention_mask, Wq, Wk, Wv, A, Wdt, Wo):
    eye_bf = np.eye(P, dtype=BF)
    eye4 = np.eye(HPC, dtype=np.float32)
    perm = np.zeros((P, P), dtype=np.float32)
    for j in range(64):
        perm[j + 64, j] = -1.0
        perm[j, j + 64] = 1.0
    perm_bf = perm.astype(BF)
    ones1 = np.ones((1, P), dtype=np.float32)
    onescol = np.ones((P, 1), dtype=BF)

    in_maps = []
    blkkeys = []
    for c in range(NCORES):
        b, g = divmod(c, 4)
        heads = list(range(4 * g, 4 * g + 4))
        wv = np.ascontiguousarray(Wv[2 * g * D:(2 * g + 2) * D].T).astype(BF)
        wdtv = np.ascontiguousarray(
            (Wdt[heads].astype(np.float64) @ Wv.astype(np.float64))
            .T.astype(np.float32))
        xT = np.ascontiguousarray(hidden_states[b].T).astype(np.float32)
        wq = np.ascontiguousarray(
            (Wq[4 * g * D:(4 * g + 4) * D] * np.float32(SCALING)).T).astype(BF)
        wk = np.ascontiguousarray(Wk[2 * g * D:(2 * g + 2) * D].T).astype(BF)
        wo = np.ascontiguousarray(Wo[:, 4 * g * D:(4 * g + 4) * D].T).astype(BF)
        acol = A[heads].astype(np.float32).reshape(HPC, 1)
        cosT = np.ascontiguousarray(cos[b].T).astype(BF)
        sinT = np.ascontiguousarray(sin[b].T).astype(BF)
        m = attention_mask[b, 0]
        mb = m.reshape(NT, P, NT, P)
        # classify [q-tile t, key-tile j] blocks
        blk = [[None] * NT for _ in range(NT)]
        varlist = []
        for t in range(NT):
            for j in range(NT):
                blkv = mb[t, :, j, :]
                if np.all(blkv == 0):
                    blk[t][j] = "Z"
                elif np.all(blkv <= -1e30):
                    blk[t][j] = "M"
                else:
                    blk[t][j] = f"V:{len(varlist)}"
                    varlist.append(np.maximum(blkv, -BIG).T)  # transposed
        # per key-tile: first allowed q-tile; interior M -> const -BIG block
        qmin = []
        for j in range(NT):
            ts = [t for t in range(NT) if blk[t][j] != "M"]
            q0 = min(ts) if ts else NT
            qmin.append(q0)
            for t in range(q0, NT):
                if blk[t][j] == "M":
                    blk[t][j] = f"V:{len(varlist)}"
                    varlist.append(np.full((P, P), -BIG, np.float32))
        if len(varlist) > NT:
            raise NotImplementedError("too many varying mask blocks")
        varblkT = np.zeros((P, NT * P), dtype=BF)
        for vi, blkv in enumerate(varlist):
            varblkT[:, vi * P:(vi + 1) * P] = blkv.astype(BF)
        blkT = tuple(tuple(blk[t][j] for t in range(NT)) for j in range(NT))
        in_maps.append({
            "xT": xT, "wq": wq, "wk": wk, "wv": wv, "wdtv": wdtv,
            "wo": wo, "acol": acol, "cosT": cosT, "sinT": sinT,
            "varblkT": varblkT, "eye_bf": eye_bf, "eye4": eye4,
            "perm_bf": perm_bf, "ones1": ones1, "onescol": onescol,
        })
        blkkeys.append((blkT, tuple(qmin)))
    if len(set(blkkeys)) != 1:
        raise NotImplementedError("mask structure differs across batches")
    return in_maps, blkkeys[0]


def _softplus64(x):
    x = x.astype(np.float64)
    return np.log1p(np.exp(-np.abs(x))) + np.maximum(x, 0)


def _repair_rows(out, bad, inputs):
    """Recompute rows flagged bad [B, S] with faithful numpy reference math."""
    if not bad.any():
        return out
    hs = inputs["hidden_states"]; cos = inputs["cos"]; sin = inputs["sin"]
    am = inputs["attention_mask"]; Wq = inputs["Wq"]; Wk = inputs["Wk"]
    Wv = inputs["Wv"]; A = inputs["A"]; Wdt = inputs["Wdt"]; Wo = inputs["Wo"]

    def rope(x, c, s):
        x1, x2 = x[..., :D // 2], x[..., D // 2:]
        return x * c + np.concatenate([-x2, x1], axis=-1) * s

    for b in range(B):
        rows = np.where(bad[b])[0]
        if len(rows) == 0:
            continue
        x = hs[b].astype(np.float32)
        k = (x @ Wk.T).reshape(S, KV, D)
        v = (x @ Wv.T).reshape(S, KV, D)
        k = rope(k, cos[b][:, None, :], sin[b][:, None, :])
        v_flat = v.reshape(S, KV * D)
        dt = v_flat @ Wdt.T
        dyn = np.exp(A[None, :] * _softplus64(dt)).astype(np.float32).T
        kth = np.sort(dyn, axis=-1)[:, NUM_DYN - 1:NUM_DYN]
        dmask = np.where(dyn < kth, MIN, dyn).astype(np.float32)
        for s_i in rows:
            q_row = (x[s_i] @ Wq.T).reshape(H, D)
            q_row = rope(q_row, cos[b][s_i][None, :], sin[b][s_i][None, :])
            attn_row = np.zeros((H, D), dtype=np.float32)
            for h in range(H):
                kvh = h // GROUPS
                sc = ((q_row[h] @ k[:, kvh].T) * np.float32(SCALING)
                      + (dmask[h] + am[b, 0, s_i])).astype(np.float32)
                w = np.exp(sc - sc.max())
                w = (w / w.sum()).astype(np.float32)
                attn_row[h] = w @ v[:, kvh]
            out[b, s_i] = attn_row.reshape(H * D) @ Wo.T
    return out


def kernel(**inputs):
    inputs = {k: np.asarray(v) for k, v in inputs.items()}
    in_maps, blkkey = _host_prep(**inputs)
    nc = _build_program(blkkey)
    res = run_bass_kernel_spmd(nc, in_maps, list(range(NCORES)))
    out = np.zeros((B, S, HID), dtype=np.float32)
    bad = np.zeros((B, S), dtype=bool)
    for c in range(NCORES):
        b = c // 4
        out[b] += res.results[c]["outT"].T
        bad[b] |= (res.results[c]["l_out"] == 0).any(axis=0)
    bad |= ~np.isfinite(out).all(axis=2)
    out = _repair_rows(out, bad, inputs)
    return out


# revision 26
# speedup vs baseline: 1.6753x; 1.0794x over previous
"""DogeDynamicMaskAttention Trainium2 kernel (v4).

Sharding: 8 cores = 2 batches x 4 head-groups. Core c: batch b=c//4,
head-group g=c%4 -> heads [4g..4g+4), kv heads {2g, 2g+1}.

Device program (SPMD; identical program on all cores, different data):
  - dt = (Wdt@Wv) @ x as an f32r pre-pass (the kthvalue threshold needs
    near-fp32 dt); the bf16 x working set is derived on-device from the
    same f32 stream, so x is DMA'd once.
  - q/k/v projections in bf16; per-output stationary reused across 4 seq
    blocks. RoPE combines on DVE, interleaved (by emission) with the
    kthvalue bisection steps so the DVE queue never idles.
  - dyn = exp(A*softplus(dt)); exact per-head kthvalue via float-bit
    bisection (device min/max init, 25 steps).
  - attention TRANSPOSED: scores^T [keys, q] = kro_kc^T @ qkro. Exps are
    mask-free exp(qk) (causal handled by multiplicative exp-masks on the
    diagonal blocks), so they never wait on the bisection. The dynamic
    mask E = exp(dyn penalized) enters through the l/av stationaries:
    l = Ecol^T P  and  av = (E*v)^T P, which is exactly softmax(qk+dyn).
  - per-chunk software pipeline: each unit's score matmuls are emitted
    zip-interleaved with the l/av matmuls of the unit LAG back, so the
    Tensor queue is never throttled by the Scalar exp rate.
  - 1/l broadcast across partitions via gpsimd.partition_broadcast.
  - output projection interleaved per q-block round, Wo resident.
  - degenerate (fully-masked) q rows give l == 0; host detects and
    recomputes those rows faithfully in numpy.
"""
import sys
import numpy as np
import ml_dtypes

sys.path.insert(0, "/root/.axon_site/_ro/trn_rl_repo")

import concourse.bass as bass  # noqa: E402,F401
from concourse import bacc  # noqa: E402
import concourse.tile as tile  # noqa: E402
import concourse.mybir as mybir  # noqa: E402
from concourse.bass_utils import run_bass_kernel_spmd  # noqa: E402
from concourse.alu_op_type import AluOpType  # noqa: E402

F32 = mybir.dt.float32
F32R = mybir.dt.float32r
BF16 = mybir.dt.bfloat16
I32 = mybir.dt.int32
AF = mybir.ActivationFunctionType
AX = mybir.AxisListType.X

B, S, HID = 2, 2048, 2048
H, KV, D = 16, 8, 128
HPC, KVPC = 4, 2
GROUPS = H // KV
NUM_DYN = S // 2
SCALING = D ** -0.5
MIN = float(np.finfo(np.float32).min)
BIG = 1.7e38
P = 128
NT = S // P          # 16
NQ = 4
QW = S // NQ         # 512
NCORES = 8
NBIS = 25
LAG = 4
BF = ml_dtypes.bfloat16

_cache = {}


def _build_program(blkkey):
    key = ("nc", blkkey)
    if key in _cache:
        return _cache[key]
    nc = bacc.Bacc("TRN2", target_bir_lowering=False, debug=False,
                   num_devices=NCORES)
    dram = {}
    for name, shape, dt in [
            ("xT", [HID, S], F32), ("wq", [HID, HPC * D], BF16),
            ("wk", [HID, KVPC * D], BF16), ("wv", [HID, KVPC * D], BF16),
            ("wdtv", [HID, HPC], F32), ("wo", [HPC * D, HID], BF16),
            ("acol", [HPC, 1], F32), ("cosT", [D, S], BF16),
            ("sinT", [D, S], BF16), ("var01T", [P, NT * P], BF16),
            ("eye_bf", [P, P], BF16), ("eye4", [HPC, HPC], F32),
            ("perm_bf", [P, P], BF16)]:
        dram[name] = nc.dram_tensor(name, shape, dt, kind="ExternalInput").ap()
    outT_d = nc.dram_tensor("outT", [HID, S], F32, kind="ExternalOutput").ap()
    l_d = nc.dram_tensor("l_out", [HPC, S], F32, kind="ExternalOutput").ap()

    blkT, qmin_t = blkkey
    with tile.TileContext(nc) as tc:
        _emit(nc, tc, dram, outT_d, l_d, blkT, qmin_t)
    nc.compile()
    _cache[key] = nc
    return nc


def _emit(nc, tc, dram, outT_d, l_d, blkT, qmin_t):
    from contextlib import ExitStack
    from itertools import zip_longest
    ctx = ExitStack()
    consts = ctx.enter_context(tc.tile_pool(name="consts", bufs=1))

    def cst(name, shape, dt):
        t = consts.tile(shape, dt, name=f"c_{name}")
        nc.sync.dma_start(t[:], dram[name])
        return t

    eye_bf = cst("eye_bf", [P, P], BF16)
    eye4_t = cst("eye4", [HPC, HPC], F32)
    perm_t = cst("perm_bf", [P, P], BF16)
    acol_t = cst("acol", [HPC, 1], F32)
    var01_t = cst("var01T", [P, NT * P], BF16)
    cos_t = cst("cosT", [D, S], BF16)
    sin_t = cst("sinT", [D, S], BF16)
    wdtv_f = consts.tile([P, NT * HPC], F32R, name="c_wdtv")
    nc.sync.dma_start(wdtv_f[:].rearrange("p (c j) -> p c j", c=NT),
                      dram["wdtv"].bitcast(F32R)
                      .rearrange("(c p) j -> p c j", p=P))
    kthc = consts.tile([HPC, 1], F32, name="kthc")
    nc.vector.memset(kthc[:], float(NUM_DYN) - 0.5)

    act = ctx.enter_context(tc.tile_pool(name="act", bufs=1))
    qkro = [act.tile([P, S], BF16, name=f"qro{h}") for h in range(HPC)]
    kro = [act.tile([P, S], BF16, name=f"kro{i}") for i in range(KVPC)]
    vn = act.tile([P, KVPC * NT * P], BF16, name="vn")
    attnT = [act.tile([P, S], BF16, name=f"attnT{h}") for h in range(HPC)]
    ecol = act.tile([P, NT * HPC], BF16, name="ecol")
    ecol_f = act.tile([P, NT * HPC], F32, name="ecol_f")
    dynrow = act.tile([HPC, S], F32, name="dynrow")
    erow = act.tile([HPC, S], F32, name="erow")

    with ExitStack() as ctx1:
        xp = ctx1.enter_context(tc.tile_pool(name="xp", bufs=1))
        xfull = xp.tile([P, NT * S], BF16, name="xfull")
        vT = [xp.tile([P, S], BF16, name=f"vT{i}") for i in range(KVPC)]
        dt_sb = xp.tile([HPC, S], F32, name="dt_sb")
        x3p = ctx1.enter_context(tc.tile_pool(name="x3p", bufs=2))
        dyq = ctx1.enter_context(tc.tile_pool(name="dyq", bufs=1))
        wp = ctx1.enter_context(tc.tile_pool(name="wp", bufs=2))
        pjp = ctx1.enter_context(tc.tile_pool(name="pjp", bufs=4))
        rsb = ctx1.enter_context(tc.tile_pool(name="rsb", bufs=3))

        # ---------------- dt pre-pass (f32r; x streamed once) ------------
        with tc.tile_pool(name="dps", bufs=4, space="PSUM") as dps:
            dt_ps = [dps.tile([HPC, QW], F32, name="dtp", tag="dtp")
                     for _ in range(NQ)]
            for cc in range(NT):
                x32 = x3p.tile([P, S], F32R, name="x32", tag="x32")
                nc.sync.dma_start(
                    x32[:], dram["xT"].bitcast(F32R)[cc * P:(cc + 1) * P, :])
                for sg in range(NQ):
                    nc.tensor.matmul(
                        dt_ps[sg][:], wdtv_f[:, cc * HPC:(cc + 1) * HPC],
                        x32[:, sg * QW:(sg + 1) * QW],
                        start=(cc == 0), stop=(cc == NT - 1),
                        skip_group_check=True)
                nc.scalar.copy(xfull[:, cc * S:(cc + 1) * S],
                               x32[:].bitcast(F32))
            for sg in range(NQ):
                nc.scalar.copy(dt_sb[:, sg * QW:(sg + 1) * QW], dt_ps[sg][:])

        # ----- dyn + bisection step emitters (interleaved with ropes) ----
        kth_f = dyq.tile([HPC, 1], I32, name="kth_f")
        dyn_t = dyq.tile([HPC, S], F32, name="dyn_t")
        work = dyq.tile([HPC, S], F32, name="work")
        scr = work
        mn = dyq.tile([HPC, 1], F32, name="mn")
        mx = dyq.tile([HPC, 1], F32, name="mx")
        lo = dyq.tile([HPC, 1], I32, name="lo")
        hi = dyq.tile([HPC, 1], I32, name="hi")
        mid = dyq.tile([HPC, 1], I32, name="mid")
        dlt = dyq.tile([HPC, 1], I32, name="dlt")
        cges = dyq.tile([HPC, 1], I32, name="cges")
        cltv = dyq.tile([HPC, 1], I32, name="cltv")
        cnt = dyq.tile([HPC, 1], F32, name="cnt")

        def bis_steps():
            nc.scalar.activation(work[:], dt_sb[:], AF.Exp)
            nc.scalar.activation(work[:], work[:], AF.Ln, bias=1.0)
            nc.scalar.activation(dyn_t[:], work[:], AF.Exp, scale=acol_t[:])
            nc.vector.tensor_reduce(mn[:], dyn_t[:], axis=AX,
                                    op=AluOpType.min)
            nc.vector.tensor_reduce(mx[:], dyn_t[:], axis=AX,
                                    op=AluOpType.max)
            nc.vector.tensor_copy(lo[:], mn[:].bitcast(I32))
            nc.vector.tensor_scalar(hi[:], mx[:].bitcast(I32), 1, None,
                                    op0=AluOpType.add)
            yield
            for _ in range(NBIS):
                nc.vector.tensor_tensor(dlt[:], hi[:], lo[:],
                                        op=AluOpType.subtract)
                nc.vector.tensor_scalar(dlt[:], dlt[:], 1, None,
                                        op0=AluOpType.arith_shift_right)
                nc.vector.tensor_tensor(mid[:], dlt[:], lo[:],
                                        op=AluOpType.add)
                nc.vector.tensor_scalar(scr[:], dyn_t[:],
                                        mid[:, 0:1].bitcast(F32), 0.0,
                                        op0=AluOpType.is_lt,
                                        op1=AluOpType.add,
                                        accum_out=cnt[:])
                nc.vector.tensor_scalar(cges[:], kthc[:], cnt[:, 0:1],
                                        None, op0=AluOpType.is_lt)
                nc.vector.tensor_scalar(cltv[:], kthc[:], cnt[:, 0:1],
                                        None, op0=AluOpType.is_ge)
                nc.vector.copy_predicated(hi[:], cges[:], mid[:])
                nc.vector.copy_predicated(lo[:], cltv[:], mid[:])
                yield
            nc.vector.tensor_copy(kth_f[:], lo[:])
            pen = work
            nc.vector.tensor_scalar(pen[:], dyn_t[:],
                                    kth_f[:, 0:1].bitcast(F32), -BIG,
                                    op0=AluOpType.is_lt,
                                    op1=AluOpType.mult)
            nc.vector.tensor_tensor(dynrow[:], dyn_t[:], pen[:],
                                    op=AluOpType.add)
            nc.scalar.activation(erow[:], dynrow[:], AF.Exp)
            yield

        bis = bis_steps()

        def pump(n):
            for _ in range(n):
                if next(bis, "done") == "done":
                    break

        # ---------------- projections (bf16); ropes on DVE ---------------
        with tc.tile_pool(name="pps", bufs=6, space="PSUM") as pps, \
             tc.tile_pool(name="rps", bufs=2, space="PSUM") as rps:
            wname = {"v": "wv", "q": "wq", "k": "wk"}
            OT = ([("k", i) for i in range(KVPC)]
                  + [("q", i) for i in range(HPC)]
                  + [("v", i) for i in range(KVPC)])
            pump(1)
            for kind, oi in OT:
                wfull = wp.tile([P, NT * P], BF16, name="wfull", tag="wf")
                nc.gpsimd.dma_start(
                    wfull[:].rearrange("p (c f) -> p c f", c=NT),
                    dram[wname[kind]][:, oi * P:(oi + 1) * P]
                    .rearrange("(c p) f -> p c f", p=P))
                ps_sg = [pps.tile([P, QW], F32, name="ps", tag="ps")
                         for _ in range(NQ)]
                for cc in range(NT):
                    st = wfull[:, cc * P:(cc + 1) * P]
                    for sg in range(NQ):
                        nc.tensor.matmul(
                            ps_sg[sg][:], st,
                            xfull[:, cc * S + sg * QW:cc * S + (sg + 1) * QW],
                            start=(cc == 0), stop=(cc == NT - 1),
                            skip_group_check=True)
                if kind == "v":
                    for sg in range(NQ):
                        nc.scalar.copy(vT[oi][:, sg * QW:(sg + 1) * QW],
                                       ps_sg[sg][:])
                else:
                    dstro = qkro[oi] if kind == "q" else kro[oi]
                    for sg in range(NQ):
                        q_sb = pjp.tile([P, QW], BF16, name="q_sb", tag="pj")
                        nc.scalar.copy(q_sb[:], ps_sg[sg][:])
                        rh = rps.tile([P, QW], F32, name="rh", tag="rh")
                        nc.tensor.matmul(rh[:], perm_t[:], q_sb[:],
                                         start=True, stop=True,
                                         skip_group_check=True)
                        rh_sb = rsb.tile([P, QW], BF16, name="rh_sb", tag="rs")
                        nc.scalar.copy(rh_sb[:], rh[:])
                        nc.vector.tensor_tensor(
                            rh_sb[:], rh_sb[:], sin_t[:, sg * QW:(sg + 1) * QW],
                            op=AluOpType.mult)
                        nc.vector.tensor_tensor(
                            q_sb[:], q_sb[:], cos_t[:, sg * QW:(sg + 1) * QW],
                            op=AluOpType.mult)
                        nc.vector.tensor_tensor(
                            dstro[:, sg * QW:(sg + 1) * QW], rh_sb[:], q_sb[:],
                            op=AluOpType.add)
                pump(4)
            pump(NBIS)

        # v natural-layout transposes (proj PSUM pools closed)
        with tc.tile_pool(name="tps", bufs=4, space="PSUM") as tps:
            for i in range(KVPC):
                for cc in range(NT):
                    pt = tps.tile([P, P], BF16, name="vt", tag="vt")
                    nc.tensor.transpose(pt[:], vT[i][:, cc * P:(cc + 1) * P],
                                        eye_bf[:])
                    nc.scalar.copy(
                        vn[:, (i * NT + cc) * P:(i * NT + cc + 1) * P], pt[:])

    # ---------------- attention + interleaved output projection ---------
    with tc.tile_pool(name="wop", bufs=1) as wop, \
         tc.tile_pool(name="ptp", bufs=79) as ptp, \
         tc.tile_pool(name="lsp", bufs=2) as lsp, \
         tc.tile_pool(name="oub", bufs=3) as oub, \
         tc.tile_pool(name="scp", bufs=2, space="PSUM") as scp, \
         tc.tile_pool(name="ovl", bufs=2, space="PSUM") as ovl, \
         tc.tile_pool(name="lpp", bufs=2, space="PSUM") as lpp, \
         tc.tile_pool(name="etp", bufs=1, space="PSUM") as etp, \
         tc.tile_pool(name="opp", bufs=1, space="PSUM") as opp:
        wo_all = wop.tile([P, NT * HPC * P], BF16, name="wo_all")
        nc.sync.dma_start(
            wo_all[:].rearrange("p (t h f) -> p t h f", t=NT, h=HPC),
            dram["wo"].rearrange("(h p) (t f) -> p t h f", p=P, f=P))
        vne = wop.tile([P, HPC * NT * P], BF16, name="vne")

        def unit_kcs(qb):
            return [kc for kc in range(NT) if qmin_t[kc] < 4 * (qb + 1)]

        def sc_exp_emitters(h, qb, pts):
            kv = h // GROUPS
            for i, kc in enumerate(unit_kcs(qb)):
                def go(i=i, kc=kc):
                    offt = max(qmin_t[kc] - 4 * qb, 0)
                    off = offt * P
                    assert i > 0 or off == 0
                    sc = scp.tile([P, QW], F32, name="sc", tag="sc")
                    nc.tensor.matmul(
                        sc[:, off:QW],
                        kro[kv][:, kc * P:(kc + 1) * P],
                        qkro[h][:, qb * QW + off:(qb + 1) * QW],
                        start=True, stop=True, skip_group_check=True)
                    pt = ptp.tile([P, QW], BF16, name="pt", tag="pt")
                    nc.scalar.activation(pt[:, off:QW], sc[:, off:QW], AF.Exp)
                    pts.append((kc, pt, off))
                yield go

        def build_e():
            # Ecol [keys, (kc,h)] bf16 from erow via PE transposes, then
            # vne[h] = vn[kv] * E[h] per chunk (DVE).
            for cc in range(NT):
                ec = etp.tile([P, HPC], F32, name="ec", tag="ec")
                nc.tensor.transpose(ec[:], erow[:, cc * P:(cc + 1) * P],
                                    eye4_t[:])
                nc.scalar.copy(ecol_f[:, cc * HPC:(cc + 1) * HPC], ec[:])
                nc.scalar.copy(ecol[:, cc * HPC:(cc + 1) * HPC], ec[:])
            for h in range(HPC):
                kv = h // GROUPS
                for kc in range(NT):
                    nc.vector.tensor_scalar(
                        vne[:, (h * NT + kc) * P:(h * NT + kc + 1) * P],
                        vn[:, (kv * NT + kc) * P:(kv * NT + kc + 1) * P],
                        ecol_f[:, kc * HPC + h:kc * HPC + h + 1], None,
                        op0=AluOpType.mult)

        def lav_emitters(h, qb, pts, out):
            kcs = unit_kcs(qb)
            n = len(kcs)
            lp = lpp.tile([1, QW], F32, name="lp", tag="lp")
            ovp = ovl.tile([P, QW], F32, name="ovp", tag="ovp")
            out.append((lp, ovp))

            def tri(i):
                kc, pt, off = pts[i]
                offt = off // P
                for t in range(4 * qb + offt, 4 * qb + 4):
                    st = blkT[kc][t]
                    if st.startswith("V"):
                        vi = int(st[2:])
                        o2 = (t - 4 * qb) * P
                        nc.vector.tensor_tensor(
                            pt[:, o2:o2 + P], pt[:, o2:o2 + P],
                            var01_t[:, vi * P:(vi + 1) * P],
                            op=AluOpType.mult)

            for i in range(n):
                def go(i=i):
                    kc, pt, off = pts[i]
                    tri(i)
                    nc.tensor.matmul(
                        lp[:, off:QW],
                        ecol[:, kc * HPC + h:kc * HPC + h + 1],
                        pt[:, off:QW],
                        start=(i == 0), stop=(i == n - 1),
                        skip_group_check=True)
                    nc.tensor.matmul(
                        ovp[:, off:QW],
                        vne[:, (h * NT + kc) * P:(h * NT + kc + 1) * P],
                        pt[:, off:QW],
                        start=(i == 0), stop=(i == n - 1),
                        skip_group_check=True)
                yield go

        def emit_unit_tail(h, qb, lp, ovp):
            lsb = lsp.tile([1, QW], F32, name="lsb", tag="lsb")
            nc.vector.tensor_copy(lsb[:], lp[:])
            nc.sync.dma_start(l_d[h:h + 1, qb * QW:(qb + 1) * QW], lsb[:])
            linv = lsp.tile([1, QW], F32, name="linv", tag="li")
            nc.vector.reciprocal(linv[:], lp[:])
            bcast = lsp.tile([P, QW], F32, name="bcast", tag="bc")
            nc.gpsimd.partition_broadcast(bcast[:], linv[:])
            nc.vector.tensor_tensor(
                attnT[h][:, qb * QW:(qb + 1) * QW], ovp[:], bcast[:],
                op=AluOpType.mult)

        def emit_outproj(sg):
            for ht in range(NT):
                op_ps = opp.tile([P, QW], F32, name="op", tag="op")
                for h in range(HPC):
                    nc.tensor.matmul(
                        op_ps[:], wo_all[:, (ht * HPC + h) * P:
                                         (ht * HPC + h + 1) * P],
                        attnT[h][:, sg * QW:(sg + 1) * QW],
                        start=(h == 0), stop=(h == HPC - 1),
                        skip_group_check=True)
                ot = oub.tile([P, QW], F32, name="ot", tag="ot")
                nc.scalar.copy(ot[:], op_ps[:])
                nc.sync.dma_start(
                    outT_d[ht * P:(ht + 1) * P, sg * QW:(sg + 1) * QW], ot[:])

        units = [(h, qb) for qb in range(NQ) for h in range(HPC)]
        upts = {}
        ulp = {}
        built_e = False

        def complete(idx_c, sc_gen):
            h_c, qb_c = units[idx_c]
            out = []
            gen_lav = lav_emitters(h_c, qb_c, upts[idx_c], out)
            for a, b in zip_longest(sc_gen, gen_lav):
                if a:
                    a()
                if b:
                    b()
            lp, ovp = out[0]
            emit_unit_tail(h_c, qb_c, lp, ovp)
            del upts[idx_c]
            if h_c == HPC - 1:
                emit_outproj(qb_c)

        for idx, (h, qb) in enumerate(units):
            pts = []
            upts[idx] = pts
            gen_sc = sc_exp_emitters(h, qb, pts)
            if idx < LAG:
                for a in gen_sc:
                    a()
                continue
            if not built_e:
                build_e()
                built_e = True
            complete(idx - LAG, gen_sc)
        for idx_c in range(len(units) - LAG, len(units)):
            complete(idx_c, iter(()))
    ctx.close()


def _host_prep(hidden_states, cos, sin, attention_mask, Wq, Wk, Wv, A, Wdt, Wo):
    eye_bf = np.eye(P, dtype=BF)
    eye4 = np.eye(HPC, dtype=np.float32)
    perm = np.zeros((P, P), dtype=np.float32)
    for j in range(64):
        perm[j + 64, j] = -1.0
        perm[j, j + 64] = 1.0
    perm_bf = perm.astype(BF)

    in_maps = []
    blkkeys = []
    for c in range(NCORES):
        b, g = divmod(c, 4)
        heads = list(range(4 * g, 4 * g + 4))
        wv = np.ascontiguousarray(Wv[2 * g * D:(2 * g + 2) * D].T).astype(BF)
        wdtv = np.ascontiguousarray(
            (Wdt[heads].astype(np.float64) @ Wv.astype(np.float64))
            .T.astype(np.float32))
        xT = np.ascontiguousarray(hidden_states[b].T).astype(np.float32)
        wq = np.ascontiguousarray(
            (Wq[4 * g * D:(4 * g + 4) * D] * np.float32(SCALING)).T).astype(BF)
        wk = np.ascontiguousarray(Wk[2 * g * D:(2 * g + 2) * D].T).astype(BF)
        wo = np.ascontiguousarray(Wo[:, 4 * g * D:(4 * g + 4) * D].T).astype(BF)
        acol = A[heads].astype(np.float32).reshape(HPC, 1)
        cosT = np.ascontiguousarray(cos[b].T).astype(BF)
        sinT = np.ascontiguousarray(sin[b].T).astype(BF)
        m = attention_mask[b, 0]
        mb = m.reshape(NT, P, NT, P)
        # classify [q-tile t, key-tile j] blocks
        blk = [[None] * NT for _ in range(NT)]
        varlist = []
        for t in range(NT):
            for j in range(NT):
                blkv = mb[t, :, j, :]
                if np.all(blkv == 0):
                    blk[t][j] = "Z"
                elif np.all(blkv <= -1e30):
                    blk[t][j] = "M"
                else:
                    blk[t][j] = f"V:{len(varlist)}"
                    varlist.append(np.maximum(blkv, -BIG).T)  # transposed
        # per key-tile: first allowed q-tile; interior M -> const -BIG block
        qmin = []
        for j in range(NT):
            ts = [t for t in range(NT) if blk[t][j] != "M"]
            q0 = min(ts) if ts else NT
            qmin.append(q0)
            for t in range(q0, NT):
                if blk[t][j] == "M":
                    blk[t][j] = f"V:{len(varlist)}"
                    varlist.append(np.full((P, P), -BIG, np.float32))
        if len(varlist) > NT:
            raise NotImplementedError("too many varying mask blocks")
        # multiplicative masks: exp of the additive block (0 -> 1, -BIG -> 0)
        var01T = np.zeros((P, NT * P), dtype=BF)
        for vi, blkv in enumerate(varlist):
            with np.errstate(over="ignore", under="ignore"):
                var01T[:, vi * P:(vi + 1) * P] = \
                    np.exp(blkv.astype(np.float64)).astype(np.float32).astype(BF)
        blkT = tuple(tuple(blk[t][j] for t in range(NT)) for j in range(NT))
        in_maps.append({
            "xT": xT, "wq": wq, "wk": wk, "wv": wv, "wdtv": wdtv,
            "wo": wo, "acol": acol, "cosT": cosT, "sinT": sinT,
            "var01T": var01T, "eye_bf": eye_bf, "eye4": eye4,
            "perm_bf": perm_bf,
        })
        blkkeys.append((blkT, tuple(qmin)))
    if len(set(blkkeys)) != 1:
        raise NotImplementedError("mask structure differs across batches")
    return in_maps, blkkeys[0]


def _softplus64(x):
    x = x.astype(np.float64)
    return np.log1p(np.exp(-np.abs(x))) + np.maximum(x, 0)


def _repair_rows(out, bad, inputs):
    """Recompute rows flagged bad [B, S] with faithful numpy reference math."""
    if not bad.any():
        return out
    hs = inputs["hidden_states"]; cos = inputs["cos"]; sin = inputs["sin"]
    am = inputs["attention_mask"]; Wq = inputs["Wq"]; Wk = inputs["Wk"]
    Wv = inputs["Wv"]; A = inputs["A"]; Wdt = inputs["Wdt"]; Wo = inputs["Wo"]

    def rope(x, c, s):
        x1, x2 = x[..., :D // 2], x[..., D // 2:]
        return x * c + np.concatenate([-x2, x1], axis=-1) * s

    for b in range(B):
        rows = np.where(bad[b])[0]
        if len(rows) == 0:
            continue
        x = hs[b].astype(np.float32)
        k = (x @ Wk.T).reshape(S, KV, D)
        v = (x @ Wv.T).reshape(S, KV, D)
        k = rope(k, cos[b][:, None, :], sin[b][:, None, :])
        v_flat = v.reshape(S, KV * D)
        dt = v_flat @ Wdt.T
        dyn = np.exp(A[None, :] * _softplus64(dt)).astype(np.float32).T
        kth = np.sort(dyn, axis=-1)[:, NUM_DYN - 1:NUM_DYN]
        dmask = np.where(dyn < kth, MIN, dyn).astype(np.float32)
        for s_i in rows:
            q_row = (x[s_i] @ Wq.T).reshape(H, D)
            q_row = rope(q_row, cos[b][s_i][None, :], sin[b][s_i][None, :])
            attn_row = np.zeros((H, D), dtype=np.float32)
            for h in range(H):
                kvh = h // GROUPS
                sc = ((q_row[h] @ k[:, kvh].T) * np.float32(SCALING)
                      + (dmask[h] + am[b, 0, s_i])).astype(np.float32)
                w = np.exp(sc - sc.max())
                w = (w / w.sum()).astype(np.float32)
                attn_row[h] = w @ v[:, kvh]
            out[b, s_i] = attn_row.reshape(H * D) @ Wo.T
    return out


def kernel(**inputs):
    inputs = {k: np.asarray(v) for k, v in inputs.items()}
    in_maps, blkkey = _host_prep(**inputs)
    nc = _build_program(blkkey)
    res = run_bass_kernel_spmd(nc, in_maps, list(range(NCORES)))
    out = np.zeros((B, S, HID), dtype=np.float32)
    bad = np.zeros((B, S), dtype=bool)
    for c in range(NCORES):
        b = c // 4
        out[b] += res.results[c]["outT"].T
        bad[b] |= (res.results[c]["l_out"] == 0).any(axis=0)
    bad |= ~np.isfinite(out).all(axis=2)
    out = _repair_rows(out, bad, inputs)
    return out


# revision 33
# speedup vs baseline: 1.7959x; 1.0720x over previous
"""DogeDynamicMaskAttention Trainium2 kernel (v4).

Sharding: 8 cores = 2 batches x 4 head-groups. Core c: batch b=c//4,
head-group g=c%4 -> heads [4g..4g+4), kv heads {2g, 2g+1}.

Device program (SPMD; identical program on all cores, different data):
  - dt = (Wdt@Wv) @ x as an f32r pre-pass (the kthvalue threshold needs
    near-fp32 dt); the bf16 x working set is derived on-device from the
    same f32 stream, so x is DMA'd once.
  - q/k/v projections in bf16; per-output stationary reused across 4 seq
    blocks. RoPE combines on DVE, interleaved (by emission) with the
    kthvalue bisection steps so the DVE queue never idles.
  - dyn = exp(A*softplus(dt)); exact per-head kthvalue via float-bit
    bisection (device min/max init, 25 steps).
  - attention TRANSPOSED: scores^T [keys, q] = kro_kc^T @ qkro. Exps are
    mask-free exp(qk) (causal handled by multiplicative exp-masks on the
    diagonal blocks), so they never wait on the bisection. The dynamic
    mask E = exp(dyn penalized) enters through the l/av stationaries:
    l = Ecol^T P  and  av = (E*v)^T P, which is exactly softmax(qk+dyn).
  - per-chunk software pipeline: each unit's score matmuls are emitted
    zip-interleaved with the l/av matmuls of the unit LAG back, so the
    Tensor queue is never throttled by the Scalar exp rate.
  - 1/l broadcast across partitions via gpsimd.partition_broadcast.
  - output projection interleaved per q-block round, Wo resident.
  - degenerate (fully-masked) q rows give l == 0; host detects and
    recomputes those rows faithfully in numpy.
"""
import sys
import numpy as np
import ml_dtypes

sys.path.insert(0, "/root/.axon_site/_ro/trn_rl_repo")

import concourse.bass as bass  # noqa: E402,F401
from concourse import bacc  # noqa: E402
import concourse.tile as tile  # noqa: E402
import concourse.mybir as mybir  # noqa: E402
from concourse.bass_utils import run_bass_kernel_spmd  # noqa: E402
from concourse.alu_op_type import AluOpType  # noqa: E402

F32 = mybir.dt.float32
F32R = mybir.dt.float32r
BF16 = mybir.dt.bfloat16
I32 = mybir.dt.int32
AF = mybir.ActivationFunctionType
AX = mybir.AxisListType.X

B, S, HID = 2, 2048, 2048
H, KV, D = 16, 8, 128
HPC, KVPC = 4, 2
GROUPS = H // KV
NUM_DYN = S // 2
SCALING = D ** -0.5
MIN = float(np.finfo(np.float32).min)
BIG = 1.7e38
P = 128
NT = S // P          # 16
NQ = 4
QW = S // NQ         # 512
NCORES = 8
NBIS = 25
LAG = 4
BF = ml_dtypes.bfloat16

_cache = {}


def _build_program(blkkey):
    key = ("nc", blkkey)
    if key in _cache:
        return _cache[key]
    nc = bacc.Bacc("TRN2", target_bir_lowering=False, debug=False,
                   num_devices=NCORES)
    dram = {}
    for name, shape, dt in [
            ("xT", [HID, S], F32), ("wq", [HID, HPC * D], BF16),
            ("wk", [HID, KVPC * D], BF16), ("wv", [HID, KVPC * D], BF16),
            ("wdtv", [HID, HPC], F32), ("wo", [HPC * D, HID], BF16),
            ("acol", [HPC, 1], F32), ("cosT", [D, S], BF16),
            ("sinT", [D, S], BF16), ("var01T", [P, NT * P], BF16),
            ("eye_bf", [P, P], BF16), ("eye4", [HPC, HPC], F32),
            ("perm_bf", [P, P], BF16)]:
        dram[name] = nc.dram_tensor(name, shape, dt, kind="ExternalInput").ap()
    outT_d = nc.dram_tensor("outT", [HID, S], BF16,
                            kind="ExternalOutput").ap()
    l_d = nc.dram_tensor("l_out", [HPC, S], F32, kind="ExternalOutput").ap()

    blkT, qmin_t = blkkey
    with tile.TileContext(nc) as tc:
        _emit(nc, tc, dram, outT_d, l_d, blkT, qmin_t)
    nc.compile()
    _cache[key] = nc
    return nc


def _emit(nc, tc, dram, outT_d, l_d, blkT, qmin_t):
    from contextlib import ExitStack
    from itertools import zip_longest
    ctx = ExitStack()
    consts = ctx.enter_context(tc.tile_pool(name="consts", bufs=1))

    def cst(name, shape, dt):
        t = consts.tile(shape, dt, name=f"c_{name}")
        nc.sync.dma_start(t[:], dram[name])
        return t

    eye_bf = cst("eye_bf", [P, P], BF16)
    eye4_t = cst("eye4", [HPC, HPC], F32)
    perm_t = cst("perm_bf", [P, P], BF16)
    acol_t = cst("acol", [HPC, 1], F32)
    var01_t = cst("var01T", [P, NT * P], BF16)
    cos_t = cst("cosT", [D, S], BF16)
    sin_t = cst("sinT", [D, S], BF16)
    wdtv_f = consts.tile([P, NT * HPC], F32R, name="c_wdtv")
    nc.sync.dma_start(wdtv_f[:].rearrange("p (c j) -> p c j", c=NT),
                      dram["wdtv"].bitcast(F32R)
                      .rearrange("(c p) j -> p c j", p=P))
    kthc = consts.tile([HPC, 1], F32, name="kthc")
    nc.vector.memset(kthc[:], float(NUM_DYN) - 0.5)

    act = ctx.enter_context(tc.tile_pool(name="act", bufs=1))
    qkro = [act.tile([P, S], BF16, name=f"qro{h}") for h in range(HPC)]
    kro = [act.tile([P, S], BF16, name=f"kro{i}") for i in range(KVPC)]
    vn = act.tile([P, KVPC * NT * P], BF16, name="vn")
    attnT = [act.tile([P, S], BF16, name=f"attnT{h}") for h in range(HPC)]
    ecol = act.tile([P, NT * HPC], BF16, name="ecol")
    ecol_f = act.tile([P, NT * HPC], F32, name="ecol_f")
    dynrow = act.tile([HPC, S], F32, name="dynrow")
    erow = act.tile([HPC, S], F32, name="erow")

    with ExitStack() as ctx1:
        xp = ctx1.enter_context(tc.tile_pool(name="xp", bufs=1))
        xfull = xp.tile([P, NT * S], BF16, name="xfull")
        vT = [xp.tile([P, S], BF16, name=f"vT{i}") for i in range(KVPC)]
        dt_sb = xp.tile([HPC, S], F32, name="dt_sb")
        x3p = ctx1.enter_context(tc.tile_pool(name="x3p", bufs=2))
        dyq = ctx1.enter_context(tc.tile_pool(name="dyq", bufs=1))
        wp = ctx1.enter_context(tc.tile_pool(name="wp", bufs=2))
        pjp = ctx1.enter_context(tc.tile_pool(name="pjp", bufs=4))
        rsb = ctx1.enter_context(tc.tile_pool(name="rsb", bufs=3))

        # ---------------- dt pre-pass (f32r; x streamed once) ------------
        with tc.tile_pool(name="dps", bufs=4, space="PSUM") as dps:
            dt_ps = [dps.tile([HPC, QW], F32, name="dtp", tag="dtp")
                     for _ in range(NQ)]
            for cc in range(NT):
                x32 = x3p.tile([P, S], F32R, name="x32", tag="x32")
                eng = nc.sync if cc % 2 == 0 else nc.scalar
                eng.dma_start(
                    x32[:], dram["xT"].bitcast(F32R)[cc * P:(cc + 1) * P, :])
                for sg in range(NQ):
                    nc.tensor.matmul(
                        dt_ps[sg][:], wdtv_f[:, cc * HPC:(cc + 1) * HPC],
                        x32[:, sg * QW:(sg + 1) * QW],
                        start=(cc == 0), stop=(cc == NT - 1),
                        skip_group_check=True)
                nc.vector.tensor_copy(xfull[:, cc * S:(cc + 1) * S],
                                      x32[:].bitcast(F32))
            for sg in range(NQ):
                nc.scalar.copy(dt_sb[:, sg * QW:(sg + 1) * QW], dt_ps[sg][:])

        # ----- dyn + bisection step emitters (interleaved with ropes) ----
        kth_f = dyq.tile([HPC, 1], I32, name="kth_f")
        dyn_t = dyq.tile([HPC, S], F32, name="dyn_t")
        work = dyq.tile([HPC, S], F32, name="work")
        scr = work
        mn = dyq.tile([HPC, 1], F32, name="mn")
        mx = dyq.tile([HPC, 1], F32, name="mx")
        lo = dyq.tile([HPC, 1], I32, name="lo")
        hi = dyq.tile([HPC, 1], I32, name="hi")
        mid = dyq.tile([HPC, 1], I32, name="mid")
        dlt = dyq.tile([HPC, 1], I32, name="dlt")
        cges = dyq.tile([HPC, 1], I32, name="cges")
        cltv = dyq.tile([HPC, 1], I32, name="cltv")
        cnt = dyq.tile([HPC, 1], F32, name="cnt")

        def bis_steps():
            nc.scalar.activation(work[:], dt_sb[:], AF.Exp)
            nc.scalar.activation(work[:], work[:], AF.Ln, bias=1.0)
            nc.scalar.activation(dyn_t[:], work[:], AF.Exp, scale=acol_t[:])
            nc.vector.tensor_reduce(mn[:], dyn_t[:], axis=AX,
                                    op=AluOpType.min)
            nc.vector.tensor_reduce(mx[:], dyn_t[:], axis=AX,
                                    op=AluOpType.max)
            nc.vector.tensor_copy(lo[:], mn[:].bitcast(I32))
            nc.vector.tensor_scalar(hi[:], mx[:].bitcast(I32), 1, None,
                                    op0=AluOpType.add)
            yield
            for _ in range(NBIS):
                nc.vector.tensor_tensor(dlt[:], hi[:], lo[:],
                                        op=AluOpType.subtract)
                nc.vector.tensor_scalar(dlt[:], dlt[:], 1, None,
                                        op0=AluOpType.arith_shift_right)
                nc.vector.tensor_tensor(mid[:], dlt[:], lo[:],
                                        op=AluOpType.add)
                nc.vector.tensor_scalar(scr[:], dyn_t[:],
                                        mid[:, 0:1].bitcast(F32), 0.0,
                                        op0=AluOpType.is_lt,
                                        op1=AluOpType.add,
                                        accum_out=cnt[:])
                nc.vector.tensor_scalar(cges[:], kthc[:], cnt[:, 0:1],
                                        None, op0=AluOpType.is_lt)
                nc.vector.tensor_scalar(cltv[:], kthc[:], cnt[:, 0:1],
                                        None, op0=AluOpType.is_ge)
                nc.vector.copy_predicated(hi[:], cges[:], mid[:])
                nc.vector.copy_predicated(lo[:], cltv[:], mid[:])
                yield
            nc.vector.tensor_copy(kth_f[:], lo[:])
            pen = work
            nc.vector.tensor_scalar(pen[:], dyn_t[:],
                                    kth_f[:, 0:1].bitcast(F32), -BIG,
                                    op0=AluOpType.is_lt,
                                    op1=AluOpType.mult)
            nc.vector.tensor_tensor(dynrow[:], dyn_t[:], pen[:],
                                    op=AluOpType.add)
            nc.scalar.activation(erow[:], dynrow[:], AF.Exp)
            yield

        bis = bis_steps()

        def pump(n):
            for _ in range(n):
                if next(bis, "done") == "done":
                    break

        # ---------------- projections (bf16); ropes on DVE ---------------
        with tc.tile_pool(name="pps", bufs=6, space="PSUM") as pps, \
             tc.tile_pool(name="rps", bufs=2, space="PSUM") as rps:
            wname = {"v": "wv", "q": "wq", "k": "wk"}
            OT = ([("k", i) for i in range(KVPC)]
                  + [("q", i) for i in range(HPC)]
                  + [("v", i) for i in range(KVPC)])
            pump(1)
            for kind, oi in OT:
                wfull = wp.tile([P, NT * P], BF16, name="wfull", tag="wf")
                nc.gpsimd.dma_start(
                    wfull[:].rearrange("p (c f) -> p c f", c=NT),
                    dram[wname[kind]][:, oi * P:(oi + 1) * P]
                    .rearrange("(c p) f -> p c f", p=P))
                ps_sg = [pps.tile([P, QW], F32, name="ps", tag="ps")
                         for _ in range(NQ)]
                for cc in range(NT):
                    st = wfull[:, cc * P:(cc + 1) * P]
                    for sg in range(NQ):
                        nc.tensor.matmul(
                            ps_sg[sg][:], st,
                            xfull[:, cc * S + sg * QW:cc * S + (sg + 1) * QW],
                            start=(cc == 0), stop=(cc == NT - 1),
                            skip_group_check=True)
                if kind == "v":
                    for sg in range(NQ):
                        nc.scalar.copy(vT[oi][:, sg * QW:(sg + 1) * QW],
                                       ps_sg[sg][:])
                else:
                    dstro = qkro[oi] if kind == "q" else kro[oi]
                    for sg in range(NQ):
                        q_sb = pjp.tile([P, QW], BF16, name="q_sb", tag="pj")
                        nc.scalar.copy(q_sb[:], ps_sg[sg][:])
                        rh = rps.tile([P, QW], F32, name="rh", tag="rh")
                        nc.tensor.matmul(rh[:], perm_t[:], q_sb[:],
                                         start=True, stop=True,
                                         skip_group_check=True)
                        rh_sb = rsb.tile([P, QW], BF16, name="rh_sb", tag="rs")
                        nc.scalar.copy(rh_sb[:], rh[:])
                        nc.vector.tensor_tensor(
                            rh_sb[:], rh_sb[:], sin_t[:, sg * QW:(sg + 1) * QW],
                            op=AluOpType.mult)
                        nc.vector.tensor_tensor(
                            q_sb[:], q_sb[:], cos_t[:, sg * QW:(sg + 1) * QW],
                            op=AluOpType.mult)
                        nc.vector.tensor_tensor(
                            dstro[:, sg * QW:(sg + 1) * QW], rh_sb[:], q_sb[:],
                            op=AluOpType.add)
                pump(4)
            pump(NBIS)

        # v natural-layout transposes (proj PSUM pools closed)
        with tc.tile_pool(name="tps", bufs=4, space="PSUM") as tps:
            for i in range(KVPC):
                for cc in range(NT):
                    pt = tps.tile([P, P], BF16, name="vt", tag="vt")
                    nc.tensor.transpose(pt[:], vT[i][:, cc * P:(cc + 1) * P],
                                        eye_bf[:])
                    nc.scalar.copy(
                        vn[:, (i * NT + cc) * P:(i * NT + cc + 1) * P], pt[:])

    # ---------------- attention + interleaved output projection ---------
    with tc.tile_pool(name="wop", bufs=1) as wop, \
         tc.tile_pool(name="ptp", bufs=79) as ptp, \
         tc.tile_pool(name="lsp", bufs=2) as lsp, \
         tc.tile_pool(name="oub", bufs=3) as oub, \
         tc.tile_pool(name="scp", bufs=2, space="PSUM") as scp, \
         tc.tile_pool(name="ovl", bufs=2, space="PSUM") as ovl, \
         tc.tile_pool(name="lpp", bufs=2, space="PSUM") as lpp, \
         tc.tile_pool(name="opp", bufs=2, space="PSUM") as opp:
        wo_all = wop.tile([P, NT * HPC * P], BF16, name="wo_all")
        nc.sync.dma_start(
            wo_all[:].rearrange("p (t h f) -> p t h f", t=NT, h=HPC),
            dram["wo"].rearrange("(h p) (t f) -> p t h f", p=P, f=P))
        vne = wop.tile([P, HPC * NT * P], BF16, name="vne")

        def unit_kcs(qb):
            return [kc for kc in range(NT) if qmin_t[kc] < 4 * (qb + 1)]

        def sc_exp_emitters(h, qb, pts):
            kv = h // GROUPS
            for i, kc in enumerate(unit_kcs(qb)):
                def go(i=i, kc=kc):
                    offt = max(qmin_t[kc] - 4 * qb, 0)
                    off = offt * P
                    assert i > 0 or off == 0
                    sc = scp.tile([P, QW], F32, name="sc", tag="sc")
                    nc.tensor.matmul(
                        sc[:, off:QW],
                        kro[kv][:, kc * P:(kc + 1) * P],
                        qkro[h][:, qb * QW + off:(qb + 1) * QW],
                        start=True, stop=True, skip_group_check=True)
                    pt = ptp.tile([P, QW], BF16, name="pt", tag="pt")
                    nc.scalar.activation(pt[:, off:QW], sc[:, off:QW], AF.Exp)
                    pts.append((kc, pt, off))
                yield go

        def build_e():
            # Ecol [keys, (kc,h)] bf16 from erow via PE transposes, then
            # vne[h] = vn[kv] * E[h] per chunk (DVE).
            for cc in range(NT):
                ec = opp.tile([P, HPC], F32, name="ec", tag="op")
                nc.tensor.transpose(ec[:], erow[:, cc * P:(cc + 1) * P],
                                    eye4_t[:])
                nc.scalar.copy(ecol_f[:, cc * HPC:(cc + 1) * HPC], ec[:])
                nc.scalar.copy(ecol[:, cc * HPC:(cc + 1) * HPC], ec[:])
            for h in range(HPC):
                kv = h // GROUPS
                for kc in range(NT):
                    nc.vector.tensor_scalar(
                        vne[:, (h * NT + kc) * P:(h * NT + kc + 1) * P],
                        vn[:, (kv * NT + kc) * P:(kv * NT + kc + 1) * P],
                        ecol_f[:, kc * HPC + h:kc * HPC + h + 1], None,
                        op0=AluOpType.mult)

        def lav_emitters(h, qb, pts, out):
            kcs = unit_kcs(qb)
            n = len(kcs)
            lp = lpp.tile([1, QW], F32, name="lp", tag="lp")
            ovp = ovl.tile([P, QW], F32, name="ovp", tag="ovp")
            out.append((lp, ovp))

            def tri(i):
                kc, pt, off = pts[i]
                offt = off // P
                for t in range(4 * qb + offt, 4 * qb + 4):
                    st = blkT[kc][t]
                    if st.startswith("V"):
                        vi = int(st[2:])
                        o2 = (t - 4 * qb) * P
                        nc.vector.tensor_tensor(
                            pt[:, o2:o2 + P], pt[:, o2:o2 + P],
                            var01_t[:, vi * P:(vi + 1) * P],
                            op=AluOpType.mult)

            for i in range(n):
                def go(i=i):
                    kc, pt, off = pts[i]
                    tri(i)
                    nc.tensor.matmul(
                        lp[:, off:QW],
                        ecol[:, kc * HPC + h:kc * HPC + h + 1],
                        pt[:, off:QW],
                        start=(i == 0), stop=(i == n - 1),
                        skip_group_check=True)
                    nc.tensor.matmul(
                        ovp[:, off:QW],
                        vne[:, (h * NT + kc) * P:(h * NT + kc + 1) * P],
                        pt[:, off:QW],
                        start=(i == 0), stop=(i == n - 1),
                        skip_group_check=True)
                yield go

        def emit_unit_tail(h, qb, lp, ovp):
            lsb = lsp.tile([1, QW], F32, name="lsb", tag="lsb")
            nc.vector.tensor_copy(lsb[:], lp[:])
            nc.sync.dma_start(l_d[h:h + 1, qb * QW:(qb + 1) * QW], lsb[:])
            linv = lsp.tile([1, QW], F32, name="linv", tag="li")
            nc.vector.reciprocal(linv[:], lp[:])
            bcast = lsp.tile([P, QW], F32, name="bcast", tag="bc")
            nc.gpsimd.partition_broadcast(bcast[:], linv[:])
            nc.vector.tensor_tensor(
                attnT[h][:, qb * QW:(qb + 1) * QW], ovp[:], bcast[:],
                op=AluOpType.mult)

        def emit_outproj(sg):
            for ht in range(NT):
                op_ps = opp.tile([P, QW], F32, name="op", tag="op")
                for h in range(HPC):
                    nc.tensor.matmul(
                        op_ps[:], wo_all[:, (ht * HPC + h) * P:
                                         (ht * HPC + h + 1) * P],
                        attnT[h][:, sg * QW:(sg + 1) * QW],
                        start=(h == 0), stop=(h == HPC - 1),
                        skip_group_check=True)
                ot = oub.tile([P, QW], BF16, name="ot", tag="ot")
                nc.vector.tensor_copy(ot[:], op_ps[:])
                nc.sync.dma_start(
                    outT_d[ht * P:(ht + 1) * P, sg * QW:(sg + 1) * QW], ot[:])

        units = [(h, qb) for qb in range(NQ) for h in range(HPC)]
        upts = {}
        ulp = {}
        built_e = False

        def complete(idx_c, sc_gen):
            h_c, qb_c = units[idx_c]
            out = []
            gen_lav = lav_emitters(h_c, qb_c, upts[idx_c], out)
            for a, b in zip_longest(sc_gen, gen_lav):
                if a:
                    a()
                if b:
                    b()
            lp, ovp = out[0]
            emit_unit_tail(h_c, qb_c, lp, ovp)
            del upts[idx_c]
            if h_c == HPC - 1:
                emit_outproj(qb_c)

        for idx, (h, qb) in enumerate(units):
            pts = []
            upts[idx] = pts
            gen_sc = sc_exp_emitters(h, qb, pts)
            if idx < LAG:
                for a in gen_sc:
                    a()
                continue
            if not built_e:
                build_e()
                built_e = True
            complete(idx - LAG, gen_sc)
        for idx_c in range(len(units) - LAG, len(units)):
            complete(idx_c, iter(()))
    ctx.close()


def _host_prep(hidden_states, cos, sin, attention_mask, Wq, Wk, Wv, A, Wdt, Wo):
    eye_bf = np.eye(P, dtype=BF)
    eye4 = np.eye(HPC, dtype=np.float32)
    perm = np.zeros((P, P), dtype=np.float32)
    for j in range(64):
        perm[j + 64, j] = -1.0
        perm[j, j + 64] = 1.0
    perm_bf = perm.astype(BF)

    in_maps = []
    blkkeys = []
    for c in range(NCORES):
        b, g = divmod(c, 4)
        heads = list(range(4 * g, 4 * g + 4))
        wv = np.ascontiguousarray(Wv[2 * g * D:(2 * g + 2) * D].T).astype(BF)
        wdtv = np.ascontiguousarray(
            (Wdt[heads].astype(np.float64) @ Wv.astype(np.float64))
            .T.astype(np.float32))
        xT = np.ascontiguousarray(hidden_states[b].T).astype(np.float32)
        wq = np.ascontiguousarray(
            (Wq[4 * g * D:(4 * g + 4) * D] * np.float32(SCALING)).T).astype(BF)
        wk = np.ascontiguousarray(Wk[2 * g * D:(2 * g + 2) * D].T).astype(BF)
        wo = np.ascontiguousarray(Wo[:, 4 * g * D:(4 * g + 4) * D].T).astype(BF)
        acol = A[heads].astype(np.float32).reshape(HPC, 1)
        cosT = np.ascontiguousarray(cos[b].T).astype(BF)
        sinT = np.ascontiguousarray(sin[b].T).astype(BF)
        m = attention_mask[b, 0]
        mb = m.reshape(NT, P, NT, P)
        # classify [q-tile t, key-tile j] blocks
        blk = [[None] * NT for _ in range(NT)]
        varlist = []
        for t in range(NT):
            for j in range(NT):
                blkv = mb[t, :, j, :]
                if np.all(blkv == 0):
                    blk[t][j] = "Z"
                elif np.all(blkv <= -1e30):
                    blk[t][j] = "M"
                else:
                    blk[t][j] = f"V:{len(varlist)}"
                    varlist.append(np.maximum(blkv, -BIG).T)  # transposed
        # per key-tile: first allowed q-tile; interior M -> const -BIG block
        qmin = []
        for j in range(NT):
            ts = [t for t in range(NT) if blk[t][j] != "M"]
            q0 = min(ts) if ts else NT
            qmin.append(q0)
            for t in range(q0, NT):
                if blk[t][j] == "M":
                    blk[t][j] = f"V:{len(varlist)}"
                    varlist.append(np.full((P, P), -BIG, np.float32))
        if len(varlist) > NT:
            raise NotImplementedError("too many varying mask blocks")
        # multiplicative masks: exp of the additive block (0 -> 1, -BIG -> 0)
        var01T = np.zeros((P, NT * P), dtype=BF)
        for vi, blkv in enumerate(varlist):
            with np.errstate(over="ignore", under="ignore"):
                var01T[:, vi * P:(vi + 1) * P] = \
                    np.exp(blkv.astype(np.float64)).astype(np.float32).astype(BF)
        blkT = tuple(tuple(blk[t][j] for t in range(NT)) for j in range(NT))
        in_maps.append({
            "xT": xT, "wq": wq, "wk": wk, "wv": wv, "wdtv": wdtv,
            "wo": wo, "acol": acol, "cosT": cosT, "sinT": sinT,
            "var01T": var01T, "eye_bf": eye_bf, "eye4": eye4,
            "perm_bf": perm_bf,
        })
        blkkeys.append((blkT, tuple(qmin)))
    if len(set(blkkeys)) != 1:
        raise NotImplementedError("mask structure differs across batches")
    return in_maps, blkkeys[0]


def _softplus64(x):
    x = x.astype(np.float64)
    return np.log1p(np.exp(-np.abs(x))) + np.maximum(x, 0)


def _repair_rows(out, bad, inputs):
    """Recompute rows flagged bad [B, S] with faithful numpy reference math."""
    if not bad.any():
        return out
    hs = inputs["hidden_states"]; cos = inputs["cos"]; sin = inputs["sin"]
    am = inputs["attention_mask"]; Wq = inputs["Wq"]; Wk = inputs["Wk"]
    Wv = inputs["Wv"]; A = inputs["A"]; Wdt = inputs["Wdt"]; Wo = inputs["Wo"]

    def rope(x, c, s):
        x1, x2 = x[..., :D // 2], x[..., D // 2:]
        return x * c + np.concatenate([-x2, x1], axis=-1) * s

    for b in range(B):
        rows = np.where(bad[b])[0]
        if len(rows) == 0:
            continue
        x = hs[b].astype(np.float32)
        k = (x @ Wk.T).reshape(S, KV, D)
        v = (x @ Wv.T).reshape(S, KV, D)
        k = rope(k, cos[b][:, None, :], sin[b][:, None, :])
        v_flat = v.reshape(S, KV * D)
        dt = v_flat @ Wdt.T
        dyn = np.exp(A[None, :] * _softplus64(dt)).astype(np.float32).T
        kth = np.sort(dyn, axis=-1)[:, NUM_DYN - 1:NUM_DYN]
        dmask = np.where(dyn < kth, MIN, dyn).astype(np.float32)
        for s_i in rows:
            q_row = (x[s_i] @ Wq.T).reshape(H, D)
            q_row = rope(q_row, cos[b][s_i][None, :], sin[b][s_i][None, :])
            attn_row = np.zeros((H, D), dtype=np.float32)
            for h in range(H):
                kvh = h // GROUPS
                sc = ((q_row[h] @ k[:, kvh].T) * np.float32(SCALING)
                      + (dmask[h] + am[b, 0, s_i])).astype(np.float32)
                w = np.exp(sc - sc.max())
                w = (w / w.sum()).astype(np.float32)
                attn_row[h] = w @ v[:, kvh]
            out[b, s_i] = attn_row.reshape(H * D) @ Wo.T
    return out


def kernel(**inputs):
    inputs = {k: np.asarray(v) for k, v in inputs.items()}
    in_maps, blkkey = _host_prep(**inputs)
    nc = _build_program(blkkey)
    res = run_bass_kernel_spmd(nc, in_maps, list(range(NCORES)))
    out = np.zeros((B, S, HID), dtype=np.float32)
    bad = np.zeros((B, S), dtype=bool)
    for c in range(NCORES):
        b = c // 4
        out[b] += res.results[c]["outT"].T.astype(np.float32)
        bad[b] |= (res.results[c]["l_out"] == 0).any(axis=0)
    bad |= ~np.isfinite(out).all(axis=2)
    out = _repair_rows(out, bad, inputs)
    return out


# revision 37
# speedup vs baseline: 1.8004x; 1.0025x over previous
"""DogeDynamicMaskAttention Trainium2 kernel (v4).

Sharding: 8 cores = 2 batches x 4 head-groups. Core c: batch b=c//4,
head-group g=c%4 -> heads [4g..4g+4), kv heads {2g, 2g+1}.

Device program (SPMD; identical program on all cores, different data):
  - dt = (Wdt@Wv) @ x as an f32r pre-pass (the kthvalue threshold needs
    near-fp32 dt); the bf16 x working set is derived on-device from the
    same f32 stream, so x is DMA'd once.
  - q/k/v projections in bf16; per-output stationary reused across 4 seq
    blocks. RoPE combines on DVE, interleaved (by emission) with the
    kthvalue bisection steps so the DVE queue never idles.
  - dyn = exp(A*softplus(dt)); exact per-head kthvalue via float-bit
    bisection (device min/max init, 25 steps).
  - attention TRANSPOSED: scores^T [keys, q] = kro_kc^T @ qkro. Exps are
    mask-free exp(qk) (causal handled by multiplicative exp-masks on the
    diagonal blocks), so they never wait on the bisection. The dynamic
    mask E = exp(dyn penalized) enters through the l/av stationaries:
    l = Ecol^T P  and  av = (E*v)^T P, which is exactly softmax(qk+dyn).
  - per-chunk software pipeline: each unit's score matmuls are emitted
    zip-interleaved with the l/av matmuls of the unit LAG back, so the
    Tensor queue is never throttled by the Scalar exp rate.
  - 1/l broadcast across partitions via gpsimd.partition_broadcast.
  - output projection interleaved per q-block round, Wo resident.
  - degenerate (fully-masked) q rows give l == 0; host detects and
    recomputes those rows faithfully in numpy.
"""
import sys
import numpy as np
import ml_dtypes

sys.path.insert(0, "/root/.axon_site/_ro/trn_rl_repo")

import concourse.bass as bass  # noqa: E402,F401
from concourse import bacc  # noqa: E402
import concourse.tile as tile  # noqa: E402
import concourse.mybir as mybir  # noqa: E402
from concourse.bass_utils import run_bass_kernel_spmd  # noqa: E402
from concourse.alu_op_type import AluOpType  # noqa: E402

F32 = mybir.dt.float32
F32R = mybir.dt.float32r
BF16 = mybir.dt.bfloat16
I32 = mybir.dt.int32
AF = mybir.ActivationFunctionType
AX = mybir.AxisListType.X

B, S, HID = 2, 2048, 2048
H, KV, D = 16, 8, 128
HPC, KVPC = 4, 2
GROUPS = H // KV
NUM_DYN = S // 2
SCALING = D ** -0.5
MIN = float(np.finfo(np.float32).min)
BIG = 1.7e38
P = 128
NT = S // P          # 16
NQ = 4
QW = S // NQ         # 512
NCORES = 8
NBIS = 25
LAG = 4
BF = ml_dtypes.bfloat16

_cache = {}


def _build_program(blkkey):
    key = ("nc", blkkey)
    if key in _cache:
        return _cache[key]
    nc = bacc.Bacc("TRN2", target_bir_lowering=False, debug=False,
                   num_devices=NCORES)
    dram = {}
    for name, shape, dt in [
            ("xT", [HID, S], F32), ("wq", [HID, HPC * D], BF16),
            ("wk", [HID, KVPC * D], BF16), ("wv", [HID, KVPC * D], BF16),
            ("wdtv", [HID, HPC], F32), ("wo", [HPC * D, HID], BF16),
            ("acol", [HPC, 1], F32), ("cosT", [D, S], BF16),
            ("sinT", [D, S], BF16), ("var01T", [P, NT * P], BF16),
            ("eye_bf", [P, P], BF16), ("eye4", [HPC, HPC], F32),
            ("perm_bf", [P, P], BF16)]:
        dram[name] = nc.dram_tensor(name, shape, dt, kind="ExternalInput").ap()
    outT_d = nc.dram_tensor("outT", [HID, S], BF16,
                            kind="ExternalOutput").ap()
    l_d = nc.dram_tensor("l_out", [HPC, S], F32, kind="ExternalOutput").ap()

    blkT, qmin_t = blkkey
    with tile.TileContext(nc) as tc:
        _emit(nc, tc, dram, outT_d, l_d, blkT, qmin_t)
    nc.compile()
    _cache[key] = nc
    return nc


def _emit(nc, tc, dram, outT_d, l_d, blkT, qmin_t):
    from contextlib import ExitStack
    from itertools import zip_longest
    ctx = ExitStack()
    consts = ctx.enter_context(tc.tile_pool(name="consts", bufs=1))

    def cst(name, shape, dt):
        t = consts.tile(shape, dt, name=f"c_{name}")
        nc.gpsimd.dma_start(t[:], dram[name])
        return t

    # wdtv first on the gpsimd queue: the dt matmuls need it immediately
    wdtv_f = consts.tile([P, NT * HPC], F32R, name="c_wdtv")
    nc.gpsimd.dma_start(wdtv_f[:].rearrange("p (c j) -> p c j", c=NT),
                        dram["wdtv"].bitcast(F32R)
                        .rearrange("(c p) j -> p c j", p=P))
    eye_bf = cst("eye_bf", [P, P], BF16)
    eye4_t = cst("eye4", [HPC, HPC], F32)
    perm_t = cst("perm_bf", [P, P], BF16)
    acol_t = cst("acol", [HPC, 1], F32)
    var01_t = cst("var01T", [P, NT * P], BF16)
    cos_t = cst("cosT", [D, S], BF16)
    sin_t = cst("sinT", [D, S], BF16)
    kthc = consts.tile([HPC, 1], F32, name="kthc")
    nc.vector.memset(kthc[:], float(NUM_DYN) - 0.5)

    act = ctx.enter_context(tc.tile_pool(name="act", bufs=1))
    qkro = [act.tile([P, S], BF16, name=f"qro{h}") for h in range(HPC)]
    kro = [act.tile([P, S], BF16, name=f"kro{i}") for i in range(KVPC)]
    vn = act.tile([P, KVPC * NT * P], BF16, name="vn")
    attnT = [act.tile([P, S], BF16, name=f"attnT{h}") for h in range(HPC)]
    ecol = act.tile([P, NT * HPC], BF16, name="ecol")
    ecol_f = act.tile([P, NT * HPC], F32, name="ecol_f")
    dynrow = act.tile([HPC, S], F32, name="dynrow")
    erow = act.tile([HPC, S], F32, name="erow")

    with ExitStack() as ctx1:
        xp = ctx1.enter_context(tc.tile_pool(name="xp", bufs=1))
        xfull = xp.tile([P, NT * S], BF16, name="xfull")
        vT = [xp.tile([P, S], BF16, name=f"vT{i}") for i in range(KVPC)]
        dt_sb = xp.tile([HPC, S], F32, name="dt_sb")
        x3p = ctx1.enter_context(tc.tile_pool(name="x3p", bufs=4))
        dyq = ctx1.enter_context(tc.tile_pool(name="dyq", bufs=1))
        wp = ctx1.enter_context(tc.tile_pool(name="wp", bufs=2))
        pjp = ctx1.enter_context(tc.tile_pool(name="pjp", bufs=4))
        rsb = ctx1.enter_context(tc.tile_pool(name="rsb", bufs=3))

        # ---------------- dt pre-pass (f32r; x streamed once) ------------
        with tc.tile_pool(name="dps", bufs=4, space="PSUM") as dps:
            dt_ps = [dps.tile([HPC, QW], F32, name="dtp", tag="dtp")
                     for _ in range(NQ)]
            for cc in range(NT):
                for hf in range(2):
                    x32 = x3p.tile([P, S // 2], F32R, name="x32", tag="x32")
                    eng = nc.sync if hf == 0 else nc.scalar
                    eng.dma_start(
                        x32[:], dram["xT"].bitcast(F32R)
                        [cc * P:(cc + 1) * P,
                         hf * (S // 2):(hf + 1) * (S // 2)])
                    for sg in range(2 * hf, 2 * hf + 2):
                        nc.tensor.matmul(
                            dt_ps[sg][:],
                            wdtv_f[:, cc * HPC:(cc + 1) * HPC],
                            x32[:, (sg - 2 * hf) * QW:
                                (sg - 2 * hf + 1) * QW],
                            start=(cc == 0), stop=(cc == NT - 1),
                            skip_group_check=True)
                    nc.vector.tensor_copy(
                        xfull[:, cc * S + hf * (S // 2):
                              cc * S + (hf + 1) * (S // 2)],
                        x32[:].bitcast(F32))
            for sg in range(NQ):
                nc.scalar.copy(dt_sb[:, sg * QW:(sg + 1) * QW], dt_ps[sg][:])

        # ----- dyn + bisection step emitters (interleaved with ropes) ----
        kth_f = dyq.tile([HPC, 1], I32, name="kth_f")
        dyn_t = dyq.tile([HPC, S], F32, name="dyn_t")
        work = dyq.tile([HPC, S], F32, name="work")
        scr = work
        mn = dyq.tile([HPC, 1], F32, name="mn")
        mx = dyq.tile([HPC, 1], F32, name="mx")
        lo = dyq.tile([HPC, 1], I32, name="lo")
        hi = dyq.tile([HPC, 1], I32, name="hi")
        mid = dyq.tile([HPC, 1], I32, name="mid")
        dlt = dyq.tile([HPC, 1], I32, name="dlt")
        cges = dyq.tile([HPC, 1], I32, name="cges")
        cltv = dyq.tile([HPC, 1], I32, name="cltv")
        cnt = dyq.tile([HPC, 1], F32, name="cnt")

        def bis_steps():
            nc.scalar.activation(work[:], dt_sb[:], AF.Exp)
            nc.scalar.activation(work[:], work[:], AF.Ln, bias=1.0)
            nc.scalar.activation(dyn_t[:], work[:], AF.Exp, scale=acol_t[:])
            nc.vector.tensor_reduce(mn[:], dyn_t[:], axis=AX,
                                    op=AluOpType.min)
            nc.vector.tensor_reduce(mx[:], dyn_t[:], axis=AX,
                                    op=AluOpType.max)
            nc.vector.tensor_copy(lo[:], mn[:].bitcast(I32))
            nc.vector.tensor_scalar(hi[:], mx[:].bitcast(I32), 1, None,
                                    op0=AluOpType.add)
            yield
            for _ in range(NBIS):
                nc.vector.tensor_tensor(dlt[:], hi[:], lo[:],
                                        op=AluOpType.subtract)
                nc.vector.tensor_scalar(dlt[:], dlt[:], 1, None,
                                        op0=AluOpType.arith_shift_right)
                nc.vector.tensor_tensor(mid[:], dlt[:], lo[:],
                                        op=AluOpType.add)
                nc.vector.tensor_scalar(scr[:], dyn_t[:],
                                        mid[:, 0:1].bitcast(F32), 0.0,
                                        op0=AluOpType.is_lt,
                                        op1=AluOpType.add,
                                        accum_out=cnt[:])
                nc.vector.tensor_scalar(cges[:], kthc[:], cnt[:, 0:1],
                                        None, op0=AluOpType.is_lt)
                nc.vector.tensor_scalar(cltv[:], kthc[:], cnt[:, 0:1],
                                        None, op0=AluOpType.is_ge)
                nc.vector.copy_predicated(hi[:], cges[:], mid[:])
                nc.vector.copy_predicated(lo[:], cltv[:], mid[:])
                yield
            nc.vector.tensor_copy(kth_f[:], lo[:])
            pen = work
            nc.vector.tensor_scalar(pen[:], dyn_t[:],
                                    kth_f[:, 0:1].bitcast(F32), -BIG,
                                    op0=AluOpType.is_lt,
                                    op1=AluOpType.mult)
            nc.vector.tensor_tensor(dynrow[:], dyn_t[:], pen[:],
                                    op=AluOpType.add)
            nc.scalar.activation(erow[:], dynrow[:], AF.Exp)
            yield

        bis = bis_steps()

        def pump(n):
            for _ in range(n):
                if next(bis, "done") == "done":
                    break

        # ---------------- projections (bf16); ropes on DVE ---------------
        with tc.tile_pool(name="pps", bufs=6, space="PSUM") as pps, \
             tc.tile_pool(name="rps", bufs=2, space="PSUM") as rps:
            wname = {"v": "wv", "q": "wq", "k": "wk"}
            OT = ([("k", i) for i in range(KVPC)]
                  + [("q", i) for i in range(HPC)]
                  + [("v", i) for i in range(KVPC)])
            pump(1)
            for kind, oi in OT:
                wfull = wp.tile([P, NT * P], BF16, name="wfull", tag="wf")
                nc.gpsimd.dma_start(
                    wfull[:].rearrange("p (c f) -> p c f", c=NT),
                    dram[wname[kind]][:, oi * P:(oi + 1) * P]
                    .rearrange("(c p) f -> p c f", p=P))
                ps_sg = [pps.tile([P, QW], F32, name="ps", tag="ps")
                         for _ in range(NQ)]
                for cc in range(NT):
                    st = wfull[:, cc * P:(cc + 1) * P]
                    for sg in range(NQ):
                        nc.tensor.matmul(
                            ps_sg[sg][:], st,
                            xfull[:, cc * S + sg * QW:cc * S + (sg + 1) * QW],
                            start=(cc == 0), stop=(cc == NT - 1),
                            skip_group_check=True)
                if kind == "v":
                    for sg in range(NQ):
                        nc.scalar.copy(vT[oi][:, sg * QW:(sg + 1) * QW],
                                       ps_sg[sg][:])
                else:
                    dstro = qkro[oi] if kind == "q" else kro[oi]
                    for sg in range(NQ):
                        q_sb = pjp.tile([P, QW], BF16, name="q_sb", tag="pj")
                        nc.scalar.copy(q_sb[:], ps_sg[sg][:])
                        rh = rps.tile([P, QW], F32, name="rh", tag="rh")
                        nc.tensor.matmul(rh[:], perm_t[:], q_sb[:],
                                         start=True, stop=True,
                                         skip_group_check=True)
                        rh_sb = rsb.tile([P, QW], BF16, name="rh_sb", tag="rs")
                        nc.scalar.copy(rh_sb[:], rh[:])
                        nc.vector.tensor_tensor(
                            rh_sb[:], rh_sb[:], sin_t[:, sg * QW:(sg + 1) * QW],
                            op=AluOpType.mult)
                        nc.vector.tensor_tensor(
                            q_sb[:], q_sb[:], cos_t[:, sg * QW:(sg + 1) * QW],
                            op=AluOpType.mult)
                        nc.vector.tensor_tensor(
                            dstro[:, sg * QW:(sg + 1) * QW], rh_sb[:], q_sb[:],
                            op=AluOpType.add)
                pump(4)
            pump(NBIS)

        # v natural-layout transposes (proj PSUM pools closed)
        with tc.tile_pool(name="tps", bufs=4, space="PSUM") as tps:
            for i in range(KVPC):
                for cc in range(NT):
                    pt = tps.tile([P, P], BF16, name="vt", tag="vt")
                    nc.tensor.transpose(pt[:], vT[i][:, cc * P:(cc + 1) * P],
                                        eye_bf[:])
                    nc.scalar.copy(
                        vn[:, (i * NT + cc) * P:(i * NT + cc + 1) * P], pt[:])

    # ---------------- attention + interleaved output projection ---------
    with tc.tile_pool(name="wop", bufs=1) as wop, \
         tc.tile_pool(name="ptp", bufs=79) as ptp, \
         tc.tile_pool(name="lsp", bufs=2) as lsp, \
         tc.tile_pool(name="oub", bufs=3) as oub, \
         tc.tile_pool(name="scp", bufs=2, space="PSUM") as scp, \
         tc.tile_pool(name="ovl", bufs=2, space="PSUM") as ovl, \
         tc.tile_pool(name="lpp", bufs=2, space="PSUM") as lpp, \
         tc.tile_pool(name="opp", bufs=2, space="PSUM") as opp:
        wo_all = wop.tile([P, NT * HPC * P], BF16, name="wo_all")
        nc.sync.dma_start(
            wo_all[:].rearrange("p (t h f) -> p t h f", t=NT, h=HPC),
            dram["wo"].rearrange("(h p) (t f) -> p t h f", p=P, f=P))
        vne = wop.tile([P, HPC * NT * P], BF16, name="vne")

        def unit_kcs(qb):
            return [kc for kc in range(NT) if qmin_t[kc] < 4 * (qb + 1)]

        def sc_exp_emitters(h, qb, pts):
            kv = h // GROUPS
            for i, kc in enumerate(unit_kcs(qb)):
                def go(i=i, kc=kc):
                    offt = max(qmin_t[kc] - 4 * qb, 0)
                    off = offt * P
                    assert i > 0 or off == 0
                    sc = scp.tile([P, QW], F32, name="sc", tag="sc")
                    nc.tensor.matmul(
                        sc[:, off:QW],
                        kro[kv][:, kc * P:(kc + 1) * P],
                        qkro[h][:, qb * QW + off:(qb + 1) * QW],
                        start=True, stop=True, skip_group_check=True)
                    pt = ptp.tile([P, QW], BF16, name="pt", tag="pt")
                    nc.scalar.activation(pt[:, off:QW], sc[:, off:QW], AF.Exp)
                    pts.append((kc, pt, off))
                yield go

        def build_e():
            # Ecol [keys, (kc,h)] bf16 from erow via PE transposes, then
            # vne[h] = vn[kv] * E[h] per chunk (DVE).
            for cc in range(NT):
                ec = opp.tile([P, HPC], F32, name="ec", tag="op")
                nc.tensor.transpose(ec[:], erow[:, cc * P:(cc + 1) * P],
                                    eye4_t[:])
                nc.scalar.copy(ecol_f[:, cc * HPC:(cc + 1) * HPC], ec[:])
                nc.scalar.copy(ecol[:, cc * HPC:(cc + 1) * HPC], ec[:])
            for h in range(HPC):
                kv = h // GROUPS
                for kc in range(NT):
                    nc.vector.tensor_scalar(
                        vne[:, (h * NT + kc) * P:(h * NT + kc + 1) * P],
                        vn[:, (kv * NT + kc) * P:(kv * NT + kc + 1) * P],
                        ecol_f[:, kc * HPC + h:kc * HPC + h + 1], None,
                        op0=AluOpType.mult)

        def lav_emitters(h, qb, pts, out):
            kcs = unit_kcs(qb)
            n = len(kcs)
            lp = lpp.tile([1, QW], F32, name="lp", tag="lp")
            ovp = ovl.tile([P, QW], F32, name="ovp", tag="ovp")
            out.append((lp, ovp))

            def tri(i):
                kc, pt, off = pts[i]
                offt = off // P
                for t in range(4 * qb + offt, 4 * qb + 4):
                    st = blkT[kc][t]
                    if st.startswith("V"):
                        vi = int(st[2:])
                        o2 = (t - 4 * qb) * P
                        nc.vector.tensor_tensor(
                            pt[:, o2:o2 + P], pt[:, o2:o2 + P],
                            var01_t[:, vi * P:(vi + 1) * P],
                            op=AluOpType.mult)

            for i in range(n):
                def go(i=i):
                    kc, pt, off = pts[i]
                    tri(i)
                    nc.tensor.matmul(
                        lp[:, off:QW],
                        ecol[:, kc * HPC + h:kc * HPC + h + 1],
                        pt[:, off:QW],
                        start=(i == 0), stop=(i == n - 1),
                        skip_group_check=True)
                    nc.tensor.matmul(
                        ovp[:, off:QW],
                        vne[:, (h * NT + kc) * P:(h * NT + kc + 1) * P],
                        pt[:, off:QW],
                        start=(i == 0), stop=(i == n - 1),
                        skip_group_check=True)
                yield go

        def emit_unit_tail(h, qb, lp, ovp):
            lsb = lsp.tile([1, QW], F32, name="lsb", tag="lsb")
            nc.vector.tensor_copy(lsb[:], lp[:])
            nc.sync.dma_start(l_d[h:h + 1, qb * QW:(qb + 1) * QW], lsb[:])
            linv = lsp.tile([1, QW], F32, name="linv", tag="li")
            nc.vector.reciprocal(linv[:], lp[:])
            bcast = lsp.tile([P, QW], F32, name="bcast", tag="bc")
            nc.gpsimd.partition_broadcast(bcast[:], linv[:])
            nc.vector.tensor_tensor(
                attnT[h][:, qb * QW:(qb + 1) * QW], ovp[:], bcast[:],
                op=AluOpType.mult)

        def emit_outproj(sg):
            for ht in range(NT):
                op_ps = opp.tile([P, QW], F32, name="op", tag="op")
                for h in range(HPC):
                    nc.tensor.matmul(
                        op_ps[:], wo_all[:, (ht * HPC + h) * P:
                                         (ht * HPC + h + 1) * P],
                        attnT[h][:, sg * QW:(sg + 1) * QW],
                        start=(h == 0), stop=(h == HPC - 1),
                        skip_group_check=True)
                ot = oub.tile([P, QW], BF16, name="ot", tag="ot")
                nc.vector.tensor_copy(ot[:], op_ps[:])
                nc.sync.dma_start(
                    outT_d[ht * P:(ht + 1) * P, sg * QW:(sg + 1) * QW], ot[:])

        units = [(h, qb) for qb in range(NQ) for h in range(HPC)]
        upts = {}
        ulp = {}
        built_e = False

        def complete(idx_c, sc_gen):
            h_c, qb_c = units[idx_c]
            out = []
            gen_lav = lav_emitters(h_c, qb_c, upts[idx_c], out)
            for a, b in zip_longest(sc_gen, gen_lav):
                if a:
                    a()
                if b:
                    b()
            lp, ovp = out[0]
            emit_unit_tail(h_c, qb_c, lp, ovp)
            del upts[idx_c]
            if h_c == HPC - 1:
                emit_outproj(qb_c)

        for idx, (h, qb) in enumerate(units):
            pts = []
            upts[idx] = pts
            gen_sc = sc_exp_emitters(h, qb, pts)
            if idx < LAG:
                for a in gen_sc:
                    a()
                continue
            if not built_e:
                build_e()
                built_e = True
            complete(idx - LAG, gen_sc)
        for idx_c in range(len(units) - LAG, len(units)):
            complete(idx_c, iter(()))
    ctx.close()


def _host_prep(hidden_states, cos, sin, attention_mask, Wq, Wk, Wv, A, Wdt, Wo):
    eye_bf = np.eye(P, dtype=BF)
    eye4 = np.eye(HPC, dtype=np.float32)
    perm = np.zeros((P, P), dtype=np.float32)
    for j in range(64):
        perm[j + 64, j] = -1.0
        perm[j, j + 64] = 1.0
    perm_bf = perm.astype(BF)

    in_maps = []
    blkkeys = []
    for c in range(NCORES):
        b, g = divmod(c, 4)
        heads = list(range(4 * g, 4 * g + 4))
        wv = np.ascontiguousarray(Wv[2 * g * D:(2 * g + 2) * D].T).astype(BF)
        wdtv = np.ascontiguousarray(
            (Wdt[heads].astype(np.float64) @ Wv.astype(np.float64))
            .T.astype(np.float32))
        xT = np.ascontiguousarray(hidden_states[b].T).astype(np.float32)
        wq = np.ascontiguousarray(
            (Wq[4 * g * D:(4 * g + 4) * D] * np.float32(SCALING)).T).astype(BF)
        wk = np.ascontiguousarray(Wk[2 * g * D:(2 * g + 2) * D].T).astype(BF)
        wo = np.ascontiguousarray(Wo[:, 4 * g * D:(4 * g + 4) * D].T).astype(BF)
        acol = A[heads].astype(np.float32).reshape(HPC, 1)
        cosT = np.ascontiguousarray(cos[b].T).astype(BF)
        sinT = np.ascontiguousarray(sin[b].T).astype(BF)
        m = attention_mask[b, 0]
        mb = m.reshape(NT, P, NT, P)
        # classify [q-tile t, key-tile j] blocks
        blk = [[None] * NT for _ in range(NT)]
        varlist = []
        for t in range(NT):
            for j in range(NT):
                blkv = mb[t, :, j, :]
                if np.all(blkv == 0):
                    blk[t][j] = "Z"
                elif np.all(blkv <= -1e30):
                    blk[t][j] = "M"
                else:
                    blk[t][j] = f"V:{len(varlist)}"
                    varlist.append(np.maximum(blkv, -BIG).T)  # transposed
        # per key-tile: first allowed q-tile; interior M -> const -BIG block
        qmin = []
        for j in range(NT):
            ts = [t for t in range(NT) if blk[t][j] != "M"]
            q0 = min(ts) if ts else NT
            qmin.append(q0)
            for t in range(q0, NT):
                if blk[t][j] == "M":
                    blk[t][j] = f"V:{len(varlist)}"
                    varlist.append(np.full((P, P), -BIG, np.float32))
        if len(varlist) > NT:
            raise NotImplementedError("too many varying mask blocks")
        # multiplicative masks: exp of the additive block (0 -> 1, -BIG -> 0)
        var01T = np.zeros((P, NT * P), dtype=BF)
        for vi, blkv in enumerate(varlist):
            with np.errstate(over="ignore", under="ignore"):
                var01T[:, vi * P:(vi + 1) * P] = \
                    np.exp(blkv.astype(np.float64)).astype(np.float32).astype(BF)
        blkT = tuple(tuple(blk[t][j] for t in range(NT)) for j in range(NT))
        in_maps.append({
            "xT": xT, "wq": wq, "wk": wk, "wv": wv, "wdtv": wdtv,
            "wo": wo, "acol": acol, "cosT": cosT, "sinT": sinT,
            "var01T": var01T, "eye_bf": eye_bf, "eye4": eye4,
            "perm_bf": perm_bf,
        })
        blkkeys.append((blkT, tuple(qmin)))
    if len(set(blkkeys)) != 1:
        raise NotImplementedError("mask structure differs across batches")
    return in_maps, blkkeys[0]


def _softplus64(x):
    x = x.astype(np.float64)
    return np.log1p(np.exp(-np.abs(x))) + np.maximum(x, 0)


def _repair_rows(out, bad, inputs):
    """Recompute rows flagged bad [B, S] with faithful numpy reference math."""
    if not bad.any():
        return out
    hs = inputs["hidden_states"]; cos = inputs["cos"]; sin = inputs["sin"]
    am = inputs["attention_mask"]; Wq = inputs["Wq"]; Wk = inputs["Wk"]
    Wv = inputs["Wv"]; A = inputs["A"]; Wdt = inputs["Wdt"]; Wo = inputs["Wo"]

    def rope(x, c, s):
        x1, x2 = x[..., :D // 2], x[..., D // 2:]
        return x * c + np.concatenate([-x2, x1], axis=-1) * s

    for b in range(B):
        rows = np.where(bad[b])[0]
        if len(rows) == 0:
            continue
        x = hs[b].astype(np.float32)
        k = (x @ Wk.T).reshape(S, KV, D)
        v = (x @ Wv.T).reshape(S, KV, D)
        k = rope(k, cos[b][:, None, :], sin[b][:, None, :])
        v_flat = v.reshape(S, KV * D)
        dt = v_flat @ Wdt.T
        dyn = np.exp(A[None, :] * _softplus64(dt)).astype(np.float32).T
        kth = np.sort(dyn, axis=-1)[:, NUM_DYN - 1:NUM_DYN]
        dmask = np.where(dyn < kth, MIN, dyn).astype(np.float32)
        for s_i in rows:
            q_row = (x[s_i] @ Wq.T).reshape(H, D)
            q_row = rope(q_row, cos[b][s_i][None, :], sin[b][s_i][None, :])
            attn_row = np.zeros((H, D), dtype=np.float32)
            for h in range(H):
                kvh = h // GROUPS
                sc = ((q_row[h] @ k[:, kvh].T) * np.float32(SCALING)
                      + (dmask[h] + am[b, 0, s_i])).astype(np.float32)
                w = np.exp(sc - sc.max())
                w = (w / w.sum()).astype(np.float32)
                attn_row[h] = w @ v[:, kvh]
            out[b, s_i] = attn_row.reshape(H * D) @ Wo.T
    return out


def kernel(**inputs):
    inputs = {k: np.asarray(v) for k, v in inputs.items()}
    in_maps, blkkey = _host_prep(**inputs)
    nc = _build_program(blkkey)
    res = run_bass_kernel_spmd(nc, in_maps, list(range(NCORES)))
    out = np.zeros((B, S, HID), dtype=np.float32)
    bad = np.zeros((B, S), dtype=bool)
    for c in range(NCORES):
        b = c // 4
        out[b] += res.results[c]["outT"].T.astype(np.float32)
        bad[b] |= (res.results[c]["l_out"] == 0).any(axis=0)
    bad |= ~np.isfinite(out).all(axis=2)
    out = _repair_rows(out, bad, inputs)
    return out
